# revision 30
# baseline (speedup 1.0000x reference)
"""Tensor-parallel GQA attention (CustomLlamaAttention) on 8 TRN2 NeuronCores.

Sharding: heads.  Core c owns Q heads 4c..4c+3 and KV head c.
  - Wq/Wkv output dims sharded; attention fully head-local per core.
  - Output projection sharded over Wo *rows* (output dim): each core computes
    out[:, 512c:512c+512] after an AllGather of the per-core attention
    outputs (transposed layout [hd, s]) -- cheaper than the all-reduce
    variant (4MB gather vs 32MB reduce).

Per-core dataflow (f32r q/k scores at full PE rate, bf16 p/v):
  hidT [4096,2048] -> Q/KV projections (d-major matmul order so the first
  matmuls only wait on 3 DMAs) -> qT/kT [hd,s] + partial RoPE (bias copies
  split across DVE/ACT/Pool so the PSUM banks free fast at chunk
  boundaries); vT -> PE-transpose -> vN [sk,hd] bf16.

  Attention per (chunk, head), [sk,sq] layout: paired score tiles share one
  2-bank PSUM tile so a single ACT exp covers both (halves ACT per-instr
  overhead); diagonal tiles get the additive mask (pre-divided by 1/sqrt(d)
  host-side) applied only to the 128-wide diagonal block, in place in PSUM.
  Softmax denominators NEVER touch the PE: the Pool(GpSimd) engine
  accumulates exp tiles (acc += p) and a partition_all_reduce replicates
  column sums to all partitions; DVE reciprocal+multiply normalizes.

  AllGather is chunked along sq (4 collectives), each fired as soon as its
  chunk's 4 heads are normalized, hiding collective latency behind the
  remaining attention chunks and the output projection.

  Phase order: projections [3,0,1,2], attention [2,3,0,1] so the first
  attention chunk has 8 unmasked score tiles of runway while the last
  projection chunk's rope stores drain on DVE.

Matmul/DMA instructions can carry only one semaphore wait on this
toolchain (single EVENTS slot in the ISA); waitfix splits excess waits
onto sequencer NOPs.
"""

import sys

sys.path.insert(0, "/opt/trn_rl_repo")

import numpy as np

import concourse.bass as bass
import concourse.mybir as mybir
import concourse.tile as tile
from concourse import bass_isa
from concourse.bass_utils import run_bass_kernel_spmd

# ---- problem constants (hardcoded per contract) ----
B, S, H = 1, 2048, 4096
NH, NKV, HD = 32, 8, 128
ROT = 64
BASE = 10000.0
N_CORES = 8
NH_LOC = NH // N_CORES  # 4 q heads per core
QD = NH_LOC * HD  # 512 local q dims
KVD = 2 * HD  # 256 local kv dims
CH = 512  # seq chunk (psum bank width in f32)
NCH = S // CH  # 4
ND = H // 128  # 32 contraction tiles
NT = S // 128  # 16 sk tiles
SCALE = 1.0 / float(np.sqrt(HD))

# natural chunk order both phases: attention chunk 0 only needs data from
# projection chunk 0 (done first), so phase B starts with zero stall
A_ORDER = [0, 1, 2, 3]
B_ORDER = [0, 1, 2, 3]

F32 = mybir.dt.float32
F32R = mybir.dt.float32r
BF16 = mybir.dt.bfloat16
AF = mybir.ActivationFunctionType
ALU = mybir.AluOpType


# --------------------------------------------------------------------------
# waitfix: split >1 semaphore waits per instruction onto sequencer NOPs
# --------------------------------------------------------------------------
def _split_waits(nc, max_waits=1):
    isa = nc.isa
    op = isa.Opcode.NEURON_ISA_TPB_OPCODE_NOP
    n_fixed = 0
    for f in nc.m.functions:
        for blk in f.blocks:
            il = blk.instructions
            fixes = []
            for i, inst in enumerate(il):
                si = inst.sync_info
                if si is None or len(si.on_wait) <= max_waits:
                    continue
                fixes.append((i, inst))
            for i, inst in reversed(fixes):
                si = inst.sync_info
                waits = list(si.on_wait)
                keep = waits[-max_waits:]
                nops = []
                for w in waits[:-max_waits]:
                    instr, fixups = bass_isa.isa_struct(isa, op, {})
                    nop = mybir.InstISA(
                        name=nc.get_next_instruction_name(),
                        isa_opcode=op.value,
                        engine=inst.engine,
                        instr=instr,
                        op_name="NOP",
                        ins=[],
                        outs=[],
                        ant_dict={},
                        verify=True,
                        ant_isa_is_sequencer_only=True,
                        ant_sbuf_fixups=fixups or None,
                    )
                    nop.sync_info = mybir.SyncInfo(on_wait=[w], on_update=[])
                    nops.append(nop)
                inst.sync_info = mybir.SyncInfo(on_wait=keep, on_update=si.on_update)
                for j, nop in enumerate(nops):
                    il.insert(i + j, nop)
                n_fixed += 1
    return n_fixed


# --------------------------------------------------------------------------
# kernel builder (SPMD program, same for all 8 cores)
# --------------------------------------------------------------------------
def build(causal: bool, skip_collective: bool = False, skip_waitfix: bool = False) -> bass.Bass:
    nc = bass.Bass()

    hidT = nc.declare_dram_parameter("hidT", [H, S], BF16, isOutput=False)
    wq_t = nc.declare_dram_parameter("wq_t", [H, QD], BF16, isOutput=False)
    wkv_t = nc.declare_dram_parameter("wkv_t", [H, KVD], BF16, isOutput=False)
    wo_t = nc.declare_dram_parameter("wo_t", [H, QD], BF16, isOutput=False)
    bq = nc.declare_dram_parameter("bq", [128, NH_LOC], F32, isOutput=False)
    bkv = nc.declare_dram_parameter("bkv", [128, 2], F32, isOutput=False)
    bo = nc.declare_dram_parameter("bo", [1, QD], F32R, isOutput=False)
    ones_col = nc.declare_dram_parameter("ones_col", [128, 1], F32R, isOutput=False)
    ones_row = nc.declare_dram_parameter("ones_row", [1, 128], F32R, isOutput=False)
    cosT = nc.declare_dram_parameter("cosT", [ROT, S], BF16, isOutput=False)
    sinT = nc.declare_dram_parameter("sinT", [ROT, S], BF16, isOutput=False)
    ident = nc.declare_dram_parameter("ident", [128, 128], BF16, isOutput=False)
    if causal:
        # 4 diagonal mask strips: strip j is the [sk 128, sq 512] transposed
        # mask block whose diagonal offset is 128*j.  Values pre-divided by
        # SCALE (clipped to f32 range) so exp's scale multiply re-applies it.
        maskT = nc.declare_dram_parameter("maskT", [4, 128, CH], F32, isOutput=False)
    else:
        maskT = nc.declare_dram_parameter("maskT", [S, S], F32, isOutput=False)
    out = nc.declare_dram_parameter("out", [S, QD], F32, isOutput=True)

    # chunk-major layout so each sq-chunk's collective operates on a
    # contiguous DRAM region
    ag_in = nc.dram_tensor("ag_in", [NCH, QD, CH], BF16)
    ag_out = nc.dram_tensor("ag_out", [NCH, H, CH], BF16, addr_space="Shared")

    with tile.TileContext(nc) as tc:
        with tc.tile_pool(name="consts", bufs=1) as consts:
            # all const tables are DMA'd after chunk 0's first d-group so
            # they don't delay the first matmuls
            bq_t = consts.tile([128, NH_LOC], F32)
            bkv_t = consts.tile([128, 2], F32)
            ident_t = consts.tile([128, 128], BF16)
            cos_t = consts.tile([ROT, S], BF16)
            sin_t = consts.tile([ROT, S], BF16)
            bo_t = consts.tile([1, QD], F32R)
            ones_col_t = consts.tile([128, 1], F32R)
            ones_row_t = consts.tile([1, 128], F32R)
            mask_t = consts.tile([128, 4, CH], F32, name="mask_t") if causal else None

            def _load_big_consts():
                nc.sync.dma_start(out=ones_col_t, in_=ones_col[:, :])
                nc.sync.dma_start(out=ones_row_t, in_=ones_row[:, :])
                nc.sync.dma_start(out=bq_t, in_=bq[:, :])
                nc.sync.dma_start(out=bkv_t, in_=bkv[:, :])
                nc.sync.dma_start(out=cos_t, in_=cosT[:, :])
                nc.sync.dma_start(out=sin_t, in_=sinT[:, :])
                nc.sync.dma_start(out=ident_t, in_=ident[:, :])
                nc.sync.dma_start(out=bo_t, in_=bo[:, :])
                if causal:
                    nc.sync.dma_start(
                        out=mask_t, in_=maskT.rearrange("j p m -> p j m")
                    )

            with tc.tile_pool(name="qkv", bufs=1) as qkv:
                qT = qkv.tile([128, NH_LOC, S], F32R)  # [hd, head, sq]
                kT = qkv.tile([128, S], F32R)  # [hd, sk]
                vN = qkv.tile([128, NT, HD], BF16)  # [sk%128, sk tile, hd]

                # ---------------- phase A: projections -------------------
                with (
                    tc.tile_pool(name="wqkv", bufs=1) as wpool,
                    tc.tile_pool(name="hid", bufs=2) as hidp,
                    tc.tile_pool(name="ppsum", bufs=1, space="PSUM") as ppsum,
                    tc.tile_pool(name="ropetmp", bufs=2) as ropetmp,
                    tc.tile_pool(name="vtmp", bufs=1) as vtmp,
                    tc.tile_pool(name="tppsum", bufs=2, space="PSUM") as tppsum,
                ):
                    # weight tiles are loaded per-d, interleaved with the first
                    # chunk's hid tiles so the first matmuls start after ~3
                    # small DMAs instead of the full 12MB
                    wq_tile = wpool.tile([128, ND, QD], BF16)
                    wkv_tile = wpool.tile([128, ND, KVD], BF16)
                    vT = vtmp.tile([128, S], BF16)  # [hd, sk], pre-transpose

                    DG = 8
                    for ch_i, ch in enumerate(A_ORDER):
                        cs = slice(ch * CH, (ch + 1) * CH)
                        psums = [ppsum.tile([128, CH], F32, tag=f"pp{m}", name=f"pp{m}") for m in range(6)]
                        for grp in range(ND // DG):
                            if ch_i == 0 and grp == 1:
                                _load_big_consts()
                            d0 = grp * DG
                            # batched DMAs: 2 hid half-group loads + 1 per
                            # weight tensor per group (SP issue rate is the
                            # DMA bottleneck, not bandwidth)
                            ht_g = hidp.tile([128, DG, CH], BF16, tag="hid")
                            for half in range(2):
                                hd0 = d0 + half * (DG // 2)
                                nc.sync.dma_start(
                                    out=ht_g[:, half * (DG // 2) : (half + 1) * (DG // 2), :],
                                    in_=hidT[
                                        hd0 * 128 : (hd0 + DG // 2) * 128, cs
                                    ].rearrange("(g p) m -> p g m", p=128),
                                )
                            if ch_i == 0:
                                for half in range(2):
                                    hd0 = d0 + half * (DG // 2)
                                    nc.sync.dma_start(
                                        out=wq_tile[:, hd0 : hd0 + DG // 2, :],
                                        in_=wq_t[
                                            hd0 * 128 : (hd0 + DG // 2) * 128, :
                                        ].rearrange("(g p) m -> p g m", p=128),
                                    )
                                    nc.sync.dma_start(
                                        out=wkv_tile[:, hd0 : hd0 + DG // 2, :],
                                        in_=wkv_t[
                                            hd0 * 128 : (hd0 + DG // 2) * 128, :
                                        ].rearrange("(g p) m -> p g m", p=128),
                                    )
                            # d-major: the first 6 matmuls need only the first
                            # half-group hid DMA + this group's weight DMAs
                            for dl in range(DG):
                                d = d0 + dl
                                for m in range(6):
                                    if m < NH_LOC:
                                        w_ap = wq_tile[:, :, m * 128 : (m + 1) * 128]
                                    else:
                                        mm = m - NH_LOC
                                        w_ap = wkv_tile[:, :, mm * 128 : (mm + 1) * 128]
                                    nc.tensor.matmul(
                                        psums[m][:, :],
                                        w_ap[:, d, :],
                                        ht_g[:, dl, :],
                                        start=(d == 0),
                                        stop=(d == ND - 1),
                                        skip_group_check=True,
                                    )
                        # bias copies split across DVE/ACT/Pool so all 6 psums
                        # free within ~1.5us; rope rotations follow on DVE
                        nc.vector.tensor_scalar_add(qT[:, 0, cs], psums[0], bq_t[:, 0:1])
                        nc.vector.tensor_scalar_add(qT[:, 1, cs], psums[1], bq_t[:, 1:2])
                        nc.scalar.activation(qT[:, 2, cs], psums[2], AF.Identity, bias=bq_t[:, 2:3])
                        nc.scalar.activation(qT[:, 3, cs], psums[3], AF.Identity, bias=bq_t[:, 3:4])
                        nc.scalar.activation(kT[:, cs], psums[4], AF.Identity, bias=bkv_t[:, 0:1])
                        nc.scalar.activation(vT[:, cs], psums[5], AF.Identity, bias=bkv_t[:, 1:2])
                        for m in range(NH_LOC):
                            _rope_rot(nc, ropetmp, qT[:, m, cs], cos_t[:, cs], sin_t[:, cs])
                        _rope_rot(nc, ropetmp, kT[:, cs], cos_t[:, cs], sin_t[:, cs])

                        # transpose this chunk's v: vT [hd, sk] -> vN [sk, hd]
                        for tl in range(CH // 128):
                            t = ch * (CH // 128) + tl
                            pst = tppsum.tile([128, 128], BF16)
                            nc.tensor.transpose(
                                pst[:, :], vT[:, t * 128 : (t + 1) * 128], ident_t[:, :]
                            )
                            nc.vector.tensor_copy(vN[:, t, :], pst[:, :])

                # phase B + C SBUF pools open together: the wo_t load streams
                # during attention, hiding the 8MB transfer
                with (
                    tc.tile_pool(name="wo", bufs=1) as wop,
                    tc.tile_pool(name="strip", bufs=3) as stripp,
                    tc.tile_pool(name="outp", bufs=3) as outp,
                ):
                    wo_tile = wop.tile([128, ND, QD], BF16)
                    for d in range(ND):
                        nc.sync.dma_start(
                            out=wo_tile[:, d, :],
                            in_=wo_t[d * 128 : (d + 1) * 128, :],
                        )

                    # ---------------- phase B: attention ---------------------
                    with (
                        tc.tile_pool(name="sppair", bufs=2, space="PSUM") as sppair,
                        tc.tile_pool(name="spdiag", bufs=2, space="PSUM") as spdiag,
                        tc.tile_pool(name="opsum", bufs=2, space="PSUM") as opsum,
                        tc.tile_pool(name="ptile", bufs=3) as ptile,
                        tc.tile_pool(name="accp", bufs=2) as accp,
                        tc.tile_pool(name="btmp", bufs=2) as btmp,
                    ):
                        ch_order = B_ORDER if causal else list(range(NCH))
                        for ch in ch_order:
                            cs = slice(ch * CH, (ch + 1) * CH)
                            nt = 4 * (ch + 1) if causal else NT
                            n_full = nt - 4 if causal else NT
                            for h in range(NH_LOC):
                                ps_o = opsum.tile([128, CH], F32, tag="o")
                                acc = accp.tile([128, CH], F32R, tag="acc")
                                first = True
                                # full tiles, processed in pairs sharing one
                                # 2-bank psum tile -> single exp instruction
                                for pr in range(n_full // 2):
                                    t0, t1 = 2 * pr, 2 * pr + 1
                                    ps2 = sppair.tile([128, 2, CH], F32, tag="sc2")
                                    for ti, t in enumerate((t0, t1)):
                                        nc.tensor.matmul(
                                            ps2[:, ti, :],
                                            kT[:, t * 128 : (t + 1) * 128],
                                            qT[:, h, cs],
                                            start=True,
                                            stop=True,
                                        )
                                    p2 = ptile.tile([128, 2, CH], BF16, tag="p2")
                                    if causal:
                                        nc.scalar.activation(
                                            p2[:, :, :], ps2[:, :, :], AF.Exp, scale=SCALE
                                        )
                                    else:
                                        for ti, t in enumerate((t0, t1)):
                                            mt = btmp.tile([128, CH], F32, tag="mt")
                                            nc.sync.dma_start(
                                                out=mt,
                                                in_=maskT[t * 128 : (t + 1) * 128, cs],
                                            )
                                            nc.vector.scalar_tensor_tensor(
                                                ps2[:, ti, :], ps2[:, ti, :], 1.0,
                                                mt[:, :], op0=ALU.mult, op1=ALU.add,
                                            )
                                        nc.scalar.activation(
                                            p2[:, :, :], ps2[:, :, :], AF.Exp, scale=SCALE
                                        )
                                    nc.tensor.matmul(
                                        ps_o[:, :], vN[:, t0, :], p2[:, 0, :],
                                        start=first, stop=False, skip_group_check=True,
                                    )
                                    nc.tensor.matmul(
                                        ps_o[:, :], vN[:, t1, :], p2[:, 1, :],
                                        start=False, stop=(not causal and pr == n_full // 2 - 1),
                                        skip_group_check=True,
                                    )
                                    # Pool-side softmax denominator accumulation
                                    if first:
                                        nc.gpsimd.tensor_add(
                                            acc[:, :], p2[:, 0, :], p2[:, 1, :]
                                        )
                                        first = False
                                    else:
                                        nc.gpsimd.tensor_add(acc[:, :], acc[:, :], p2[:, 0, :])
                                        nc.gpsimd.tensor_add(acc[:, :], acc[:, :], p2[:, 1, :])
                                # diagonal tiles (causal only)
                                if causal:
                                    for j in range(4):
                                        t = n_full + j
                                        sq0 = 128 * j  # valid-data start
                                        sq0m = min(sq0, 256)  # f32r >=256 matmul start
                                        # chunk 0 has no pair tiles: borrow the
                                        # idle pair-pool buffers (same tag ->
                                        # same banks) for a 4-deep diag pipeline
                                        if n_full == 0 and j % 2 == 1:
                                            psd = sppair.tile([128, CH], F32, tag="sc2")
                                        else:
                                            psd = spdiag.tile([128, CH], F32, tag="scd")
                                        nc.tensor.matmul(
                                            psd[:, sq0m:],
                                            kT[:, t * 128 : (t + 1) * 128],
                                            qT[:, h, ch * CH + sq0m : (ch + 1) * CH],
                                            start=True,
                                            stop=True,
                                        )
                                        # mask only the 128-wide diagonal block,
                                        # in place (mask pre-divided by SCALE)
                                        nc.vector.scalar_tensor_tensor(
                                            psd[:, sq0 : sq0 + 128],
                                            psd[:, sq0 : sq0 + 128],
                                            1.0,
                                            mask_t[:, j, sq0 : sq0 + 128],
                                            op0=ALU.mult,
                                            op1=ALU.add,
                                        )
                                        pd = ptile.tile([128, CH], BF16, tag="pd")
                                        nc.scalar.activation(
                                            pd[:, sq0:], psd[:, sq0:], AF.Exp, scale=SCALE
                                        )
                                        nc.tensor.matmul(
                                            ps_o[:, sq0:], vN[:, t, :], pd[:, sq0:],
                                            start=first, stop=(j == 3),
                                            skip_group_check=True,
                                        )
                                        if first:
                                            nc.gpsimd.tensor_copy(acc[:, :], pd[:, :])
                                            first = False
                                        else:
                                            nc.gpsimd.tensor_add(
                                                acc[:, sq0:], acc[:, sq0:], pd[:, sq0:]
                                            )
                                # normalize: single short ones-matmul reduces
                                # the Pool-accumulated acc across partitions;
                                # reciprocal + PE broadcast + multiply.  The
                                # sums/bcast psums ride the diag pool's
                                # buffer rotation (no extra banks).
                                spool, stag = (
                                    (sppair, "sc2") if n_full > 0 else (spdiag, "scd")
                                )
                                sums = spool.tile([1, CH], F32, tag=stag, name="sums")
                                nc.tensor.matmul(
                                    sums[:, :], ones_col_t[:, :], acc[:, :],
                                    start=True, stop=True,
                                )
                                rec = accp.tile([1, CH], F32R, tag="rec")
                                with nc.allow_low_precision(reason="f32r recip"):
                                    nc.vector.reciprocal(rec[:, :], sums[:, :])
                                ps_bc = spool.tile([128, CH], F32, tag=stag, name="bcp")
                                nc.tensor.matmul(
                                    ps_bc[:, :], ones_row_t[:, :], rec[:, :],
                                    start=True, stop=True,
                                )
                                bc_sb = btmp.tile([128, CH], F32R, tag="bcs")
                                nc.vector.tensor_copy(bc_sb[:, :], ps_bc[:, :])
                                an = btmp.tile([128, CH], BF16, tag="an")
                                nc.vector.tensor_mul(an[:, :], ps_o[:, :], bc_sb[:, :])
                                nc.sync.dma_start(
                                    out=ag_in[ch, h * 128 : (h + 1) * 128, :],
                                    in_=an[:, :],
                                )
                            # fire this sq-chunk's AllGather as soon as its 4
                            # heads are done; overlaps remaining attention +
                            # output projection
                            if not skip_collective:
                                nc.gpsimd.collective_compute(
                                    "AllGather",
                                    ALU.bypass,
                                    ins=[ag_in[ch, :, :]],
                                    outs=[ag_out[ch, :, :]],
                                    replica_groups=[list(range(N_CORES))],
                                )

                    # ---------------- phase C: output projection ----------
                    with tc.tile_pool(name="copsum", bufs=2, space="PSUM") as copsum:
                        ps_bo = copsum.tile([128, QD], F32, tag="co0", name="psbo")
                        nc.tensor.matmul(
                            ps_bo[:, :], ones_row_t[:, :], bo_t[:, :],
                            start=True, stop=True,
                        )
                        bo_bc = outp.tile([128, QD], F32, tag="bo")
                        nc.vector.tensor_copy(bo_bc[:, :], ps_bo[:, :])

                        sqb_order = B_ORDER if causal else list(range(NCH))
                        for sqb in sqb_order:
                            ps_outs = [
                                copsum.tile(
                                    [128, QD], F32, tag=f"co{j}", name=f"co{j}"
                                )
                                for j in range(4)
                            ]
                            SG = 4  # strips per batched DMA
                            for dg in range(ND // SG):
                                d0 = dg * SG
                                strip = stripp.tile([128, SG, CH], BF16, tag="strip")
                                nc.sync.dma_start(
                                    out=strip,
                                    in_=ag_out[
                                        sqb, d0 * 128 : (d0 + SG) * 128, :
                                    ].rearrange("(g p) m -> p g m", p=128),
                                )
                                for dl in range(SG):
                                    d = d0 + dl
                                    for j in range(4):
                                        nc.tensor.matmul(
                                            ps_outs[j][:, :],
                                            strip[:, dl, j * 128 : (j + 1) * 128],
                                            wo_tile[:, d, :],
                                            start=(d == 0),
                                            stop=(d == ND - 1),
                                            skip_group_check=True,
                                        )
                            for j in range(4):
                                ot = outp.tile([128, QD], F32, tag="ot")
                                nc.vector.tensor_add(
                                    ot[:, :], ps_outs[j][:, :], bo_bc[:, :]
                                )
                                nc.sync.dma_start(
                                    out=out[
                                        sqb * CH + j * 128 : sqb * CH + (j + 1) * 128,
                                        :,
                                    ],
                                    in_=ot[:, :],
                                )

    if not skip_waitfix:
        _split_waits(nc)
    return nc


def _rope_rot(nc, tmp, dst, cos_s, sin_s):
    """In-place partial rope on dst[hd, s] (f32r): rows 0:64 rotated, rest pass.

    cos_s [64, CH]: cos table duplicated over both 32-row halves.
    sin_s [64, CH]: sign-folded sin: rows 0:32 = -sin, rows 32:64 = +sin, so
      new[0:64] = x[0:64]*cos_s + rot32(x[0:64])*sin_s
    with rot32 = swap of the two 32-row halves (done via SBUF->SBUF DMA,
    since compute engines cannot cross partitions).  All SBUF-only, so the
    muls/add run on the otherwise-idle Pool(GpSimd) engine, keeping DVE
    free for the attention phase that follows.
    """
    R2 = ROT // 2
    qsh = tmp.tile([ROT, CH], F32R, tag="r1")
    nc.sync.dma_start(out=qsh[0:R2, :], in_=dst[R2:ROT, :])
    nc.sync.dma_start(out=qsh[R2:ROT, :], in_=dst[0:R2, :])
    t1 = tmp.tile([ROT, CH], F32, tag="r2")
    nc.gpsimd.tensor_mul(t1[:, :], dst[0:ROT, :], cos_s[:, :])
    t2 = tmp.tile([ROT, CH], F32, tag="r3")
    nc.gpsimd.tensor_mul(t2[:, :], qsh[:, :], sin_s[:, :])
    nc.gpsimd.tensor_add(dst[0:ROT, :], t1[:, :], t2[:, :])


# --------------------------------------------------------------------------
# host side: shard, run, gather
# --------------------------------------------------------------------------
_NC_CACHE = {}


def _get_nc(causal: bool) -> bass.Bass:
    if causal not in _NC_CACHE:
        _NC_CACHE[causal] = build(causal)
    return _NC_CACHE[causal]


def _rope_tables():
    inv_freq = 1.0 / (BASE ** (np.arange(0, ROT, 2, dtype=np.float64) / ROT))
    t = np.arange(S, dtype=np.float64)
    freqs = np.outer(t, inv_freq)  # [S, 32]
    import ml_dtypes

    cos32 = np.cos(freqs).T.astype(np.float32)  # [32, S]
    sin32 = np.sin(freqs).T.astype(np.float32)
    cosT = np.concatenate([cos32, cos32], axis=0).astype(ml_dtypes.bfloat16)
    sinT = np.concatenate([-sin32, sin32], axis=0).astype(ml_dtypes.bfloat16)
    return cosT, sinT


def _check_causal(mask):
    """mask: [1,1,S,S]. True if it is exactly a causal additive mask."""
    m = mask[0, 0]
    if not (m[np.tril_indices(S)] == 0.0).all():
        return False
    iu = np.triu_indices(S, k=1)
    vals = m[iu]
    return bool((vals <= -1e30).all()) and bool((vals == vals[0]).all())


def _prescale_mask(m64):
    """Divide mask by SCALE in f64, clipped to the f32 range."""
    fmax = np.finfo(np.float32).max
    return np.clip(m64.astype(np.float64) / SCALE, -fmax, fmax).astype(np.float32)


def _make_in_maps(inputs, causal):
    import ml_dtypes
    hidden = np.asarray(inputs["hidden_states"], dtype=np.float32)
    mask = np.asarray(inputs["attention_mask"], dtype=np.float32)
    Wq = np.asarray(inputs["Wq"], dtype=np.float32)
    bq = np.asarray(inputs["bq"], dtype=np.float32)
    Wkv = np.asarray(inputs["Wkv"], dtype=np.float32)
    bkv = np.asarray(inputs["bkv"], dtype=np.float32)
    Wo = np.asarray(inputs["Wo"], dtype=np.float32)
    bo = np.asarray(inputs["bo"], dtype=np.float32)

    hidT_bf = np.ascontiguousarray(hidden[0].T).astype(ml_dtypes.bfloat16)  # [H, S]
    cosT, sinT = _rope_tables()
    ident = np.eye(128, dtype=ml_dtypes.bfloat16)
    ones_col = np.ones((128, 1), np.float32)
    ones_row = np.ones((1, 128), np.float32)

    if causal:
        # diagonal strips from the actual mask (chunk 0 is representative --
        # _check_causal guarantees the pattern is uniform along the diagonal)
        maskT = np.stack(
            [_prescale_mask(
                np.ascontiguousarray(mask[0, 0, 0:CH, 128 * j : 128 * j + 128].T))
             for j in range(4)]
        )  # [4, 128, CH]
    else:
        maskT = _prescale_mask(np.ascontiguousarray(mask[0, 0].T))  # [S, S]

    in_maps = []
    for c in range(N_CORES):
        qs = slice(c * QD, (c + 1) * QD)
        kvs = slice(c * KVD, (c + 1) * KVD)
        in_maps.append(
            {
                "hidT": hidT_bf,
                "wq_t": np.ascontiguousarray(Wq[qs, :].T).astype(ml_dtypes.bfloat16),
                "wkv_t": np.ascontiguousarray(Wkv[kvs, :].T).astype(ml_dtypes.bfloat16),
                "wo_t": np.ascontiguousarray(Wo[qs, :].T).astype(ml_dtypes.bfloat16),
                "bq": np.ascontiguousarray(bq[qs].reshape(NH_LOC, 128).T),
                "bkv": np.ascontiguousarray(
                    bkv[kvs].reshape(2, 128).T
                ),
                "bo": bo[qs].reshape(1, QD),
                "cosT": cosT,
                "sinT": sinT,
                "ident": ident,
                "ones_col": ones_col,
                "ones_row": ones_row,
                "maskT": maskT,
            }
        )
    return in_maps


def kernel(**inputs) -> np.ndarray:
    causal = _check_causal(np.asarray(inputs["attention_mask"], dtype=np.float32))
    nc = _get_nc(causal)
    in_maps = _make_in_maps(inputs, causal)
    res = run_bass_kernel_spmd(nc, in_maps, list(range(N_CORES)))
    outs = [res.results[c]["out"] for c in range(N_CORES)]  # each [S, QD]
    full = np.concatenate(outs, axis=1)  # [S, H]
    return full.reshape(B, S, H)


# revision 31
# speedup vs baseline: 1.0396x; 1.0396x over previous
"""Tensor-parallel GQA attention (CustomLlamaAttention) on 8 TRN2 NeuronCores.

Sharding: heads.  Core c owns Q heads 4c..4c+3 and KV head c.
  - Wq/Wkv output dims sharded; attention fully head-local per core.
  - Output projection sharded over Wo *rows* (output dim): each core computes
    out[:, 512c:512c+512] after an AllGather of the per-core attention
    outputs (transposed layout [hd, s]) -- cheaper than the all-reduce
    variant (4MB gather vs 32MB reduce).

Per-core dataflow (f32r q/k scores at full PE rate, bf16 p/v):
  hidT [4096,2048] -> Q/KV projections (d-major matmul order so the first
  matmuls only wait on 3 DMAs) -> qT/kT [hd,s] + partial RoPE (bias copies
  split across DVE/ACT/Pool so the PSUM banks free fast at chunk
  boundaries); vT -> PE-transpose -> vN [sk,hd] bf16.

  Attention per (chunk, head), [sk,sq] layout: paired score tiles share one
  2-bank PSUM tile so a single ACT exp covers both (halves ACT per-instr
  overhead); diagonal tiles get the additive mask (pre-divided by 1/sqrt(d)
  host-side) applied only to the 128-wide diagonal block, in place in PSUM.
  Softmax denominators NEVER touch the PE: the Pool(GpSimd) engine
  accumulates exp tiles (acc += p) and a partition_all_reduce replicates
  column sums to all partitions; DVE reciprocal+multiply normalizes.

  AllGather is chunked along sq (4 collectives), each fired as soon as its
  chunk's 4 heads are normalized, hiding collective latency behind the
  remaining attention chunks and the output projection.

  Phase order: projections [3,0,1,2], attention [2,3,0,1] so the first
  attention chunk has 8 unmasked score tiles of runway while the last
  projection chunk's rope stores drain on DVE.

Matmul/DMA instructions can carry only one semaphore wait on this
toolchain (single EVENTS slot in the ISA); waitfix splits excess waits
onto sequencer NOPs.
"""

import sys

sys.path.insert(0, "/opt/trn_rl_repo")

import numpy as np

import concourse.bass as bass
import concourse.mybir as mybir
import concourse.tile as tile
from concourse import bass_isa
from concourse.bass_utils import run_bass_kernel_spmd

# ---- problem constants (hardcoded per contract) ----
B, S, H = 1, 2048, 4096
NH, NKV, HD = 32, 8, 128
ROT = 64
BASE = 10000.0
N_CORES = 8
NH_LOC = NH // N_CORES  # 4 q heads per core
QD = NH_LOC * HD  # 512 local q dims
KVD = 2 * HD  # 256 local kv dims
CH = 512  # seq chunk (psum bank width in f32)
NCH = S // CH  # 4
ND = H // 128  # 32 contraction tiles
NT = S // 128  # 16 sk tiles
SCALE = 1.0 / float(np.sqrt(HD))

# natural chunk order both phases: attention chunk 0 only needs data from
# projection chunk 0 (done first), so phase B starts with zero stall
A_ORDER = [0, 1, 2, 3]
B_ORDER = [0, 1, 2, 3]

F32 = mybir.dt.float32
F32R = mybir.dt.float32r
BF16 = mybir.dt.bfloat16
AF = mybir.ActivationFunctionType
ALU = mybir.AluOpType


# --------------------------------------------------------------------------
# waitfix: split >1 semaphore waits per instruction onto sequencer NOPs
# --------------------------------------------------------------------------
def _split_waits(nc, max_waits=1):
    isa = nc.isa
    op = isa.Opcode.NEURON_ISA_TPB_OPCODE_NOP
    n_fixed = 0
    for f in nc.m.functions:
        for blk in f.blocks:
            il = blk.instructions
            fixes = []
            for i, inst in enumerate(il):
                si = inst.sync_info
                if si is None or len(si.on_wait) <= max_waits:
                    continue
                fixes.append((i, inst))
            for i, inst in reversed(fixes):
                si = inst.sync_info
                waits = list(si.on_wait)
                keep = waits[-max_waits:]
                nops = []
                for w in waits[:-max_waits]:
                    instr, fixups = bass_isa.isa_struct(isa, op, {})
                    nop = mybir.InstISA(
                        name=nc.get_next_instruction_name(),
                        isa_opcode=op.value,
                        engine=inst.engine,
                        instr=instr,
                        op_name="NOP",
                        ins=[],
                        outs=[],
                        ant_dict={},
                        verify=True,
                        ant_isa_is_sequencer_only=True,
                        ant_sbuf_fixups=fixups or None,
                    )
                    nop.sync_info = mybir.SyncInfo(on_wait=[w], on_update=[])
                    nops.append(nop)
                inst.sync_info = mybir.SyncInfo(on_wait=keep, on_update=si.on_update)
                for j, nop in enumerate(nops):
                    il.insert(i + j, nop)
                n_fixed += 1
    return n_fixed


# --------------------------------------------------------------------------
# kernel builder (SPMD program, same for all 8 cores)
# --------------------------------------------------------------------------
def build(causal: bool, skip_collective: bool = False, skip_waitfix: bool = False) -> bass.Bass:
    nc = bass.Bass()

    hidT = nc.declare_dram_parameter("hidT", [H, S], BF16, isOutput=False)
    wq_t = nc.declare_dram_parameter("wq_t", [H, QD], BF16, isOutput=False)
    wkv_t = nc.declare_dram_parameter("wkv_t", [H, KVD], BF16, isOutput=False)
    wo_t = nc.declare_dram_parameter("wo_t", [H, QD], BF16, isOutput=False)
    bq = nc.declare_dram_parameter("bq", [128, NH_LOC], F32, isOutput=False)
    bkv = nc.declare_dram_parameter("bkv", [128, 2], F32, isOutput=False)
    bo = nc.declare_dram_parameter("bo", [1, QD], F32R, isOutput=False)
    ones_col = nc.declare_dram_parameter("ones_col", [128, 1], F32R, isOutput=False)
    ones_row = nc.declare_dram_parameter("ones_row", [1, 128], F32R, isOutput=False)
    cosT = nc.declare_dram_parameter("cosT", [ROT, S], BF16, isOutput=False)
    sinT = nc.declare_dram_parameter("sinT", [ROT, S], BF16, isOutput=False)
    ident = nc.declare_dram_parameter("ident", [128, 128], BF16, isOutput=False)
    if causal:
        # 4 diagonal mask strips: strip j is the [sk 128, sq 512] transposed
        # mask block whose diagonal offset is 128*j.  Values pre-divided by
        # SCALE (clipped to f32 range) so exp's scale multiply re-applies it.
        maskT = nc.declare_dram_parameter("maskT", [4, 128, CH], F32, isOutput=False)
    else:
        maskT = nc.declare_dram_parameter("maskT", [S, S], F32, isOutput=False)
    out = nc.declare_dram_parameter("out", [S, QD], F32, isOutput=True)

    # chunk-major layout so each sq-chunk's collective operates on a
    # contiguous DRAM region
    ag_in = nc.dram_tensor("ag_in", [NCH, QD, CH], BF16)
    ag_out = nc.dram_tensor("ag_out", [NCH, H, CH], BF16, addr_space="Shared")

    with tile.TileContext(nc) as tc:
        with tc.tile_pool(name="consts", bufs=1) as consts:
            # all const tables are DMA'd after chunk 0's first d-group so
            # they don't delay the first matmuls
            bq_t = consts.tile([128, NH_LOC], F32)
            bkv_t = consts.tile([128, 2], F32)
            ident_t = consts.tile([128, 128], BF16)
            cos_t = consts.tile([ROT, S], BF16)
            sin_t = consts.tile([ROT, S], BF16)
            bo_t = consts.tile([1, QD], F32R)
            ones_col_t = consts.tile([128, 1], F32R)
            ones_row_t = consts.tile([1, 128], F32R)
            mask_t = consts.tile([128, 4, CH], F32, name="mask_t") if causal else None

            def _load_big_consts():
                nc.sync.dma_start(out=ones_col_t, in_=ones_col[:, :])
                nc.sync.dma_start(out=ones_row_t, in_=ones_row[:, :])
                nc.sync.dma_start(out=bq_t, in_=bq[:, :])
                nc.sync.dma_start(out=bkv_t, in_=bkv[:, :])
                nc.sync.dma_start(out=cos_t, in_=cosT[:, :])
                nc.sync.dma_start(out=sin_t, in_=sinT[:, :])
                nc.sync.dma_start(out=ident_t, in_=ident[:, :])
                nc.sync.dma_start(out=bo_t, in_=bo[:, :])
                if causal:
                    nc.sync.dma_start(
                        out=mask_t, in_=maskT.rearrange("j p m -> p j m")
                    )

            with tc.tile_pool(name="qkv", bufs=1) as qkv:
                qT = qkv.tile([128, NH_LOC, S], F32R)  # [hd, head, sq]
                kT = qkv.tile([128, S], F32R)  # [hd, sk]
                vN = qkv.tile([128, NT, HD], BF16)  # [sk%128, sk tile, hd]

                # ---------------- phase A: projections -------------------
                with (
                    tc.tile_pool(name="wqkv", bufs=1) as wpool,
                    tc.tile_pool(name="hid", bufs=2) as hidp,
                    tc.tile_pool(name="ppsum", bufs=1, space="PSUM") as ppsum,
                    tc.tile_pool(name="ropetmp", bufs=2) as ropetmp,
                    tc.tile_pool(name="vtmp", bufs=1) as vtmp,
                    tc.tile_pool(name="tppsum", bufs=2, space="PSUM") as tppsum,
                ):
                    # weight tiles are loaded per-d, interleaved with the first
                    # chunk's hid tiles so the first matmuls start after ~3
                    # small DMAs instead of the full 12MB
                    wq_tile = wpool.tile([128, ND, QD], BF16)
                    wkv_tile = wpool.tile([128, ND, KVD], BF16)
                    vT = vtmp.tile([128, S], BF16)  # [hd, sk], pre-transpose

                    DG = 8
                    for ch_i, ch in enumerate(A_ORDER):
                        cs = slice(ch * CH, (ch + 1) * CH)
                        psums = [ppsum.tile([128, CH], F32, tag=f"pp{m}", name=f"pp{m}") for m in range(6)]
                        for grp in range(ND // DG):
                            if ch_i == 0 and grp == 1:
                                _load_big_consts()
                            d0 = grp * DG
                            # batched DMAs: 2 hid half-group loads + 1 per
                            # weight tensor per group (SP issue rate is the
                            # DMA bottleneck, not bandwidth)
                            ht_g = hidp.tile([128, DG, CH], BF16, tag="hid")
                            for half in range(2):
                                hd0 = d0 + half * (DG // 2)
                                nc.sync.dma_start(
                                    out=ht_g[:, half * (DG // 2) : (half + 1) * (DG // 2), :],
                                    in_=hidT[
                                        hd0 * 128 : (hd0 + DG // 2) * 128, cs
                                    ].rearrange("(g p) m -> p g m", p=128),
                                )
                            if ch_i == 0:
                                for half in range(2):
                                    hd0 = d0 + half * (DG // 2)
                                    nc.sync.dma_start(
                                        out=wq_tile[:, hd0 : hd0 + DG // 2, :],
                                        in_=wq_t[
                                            hd0 * 128 : (hd0 + DG // 2) * 128, :
                                        ].rearrange("(g p) m -> p g m", p=128),
                                    )
                                    nc.sync.dma_start(
                                        out=wkv_tile[:, hd0 : hd0 + DG // 2, :],
                                        in_=wkv_t[
                                            hd0 * 128 : (hd0 + DG // 2) * 128, :
                                        ].rearrange("(g p) m -> p g m", p=128),
                                    )
                            # d-major: the first 6 matmuls need only the first
                            # half-group hid DMA + this group's weight DMAs
                            for dl in range(DG):
                                d = d0 + dl
                                for m in range(6):
                                    if m < NH_LOC:
                                        w_ap = wq_tile[:, :, m * 128 : (m + 1) * 128]
                                    else:
                                        mm = m - NH_LOC
                                        w_ap = wkv_tile[:, :, mm * 128 : (mm + 1) * 128]
                                    nc.tensor.matmul(
                                        psums[m][:, :],
                                        w_ap[:, d, :],
                                        ht_g[:, dl, :],
                                        start=(d == 0),
                                        stop=(d == ND - 1),
                                        skip_group_check=True,
                                    )
                        # bias copies split across DVE/ACT/Pool so all 6 psums
                        # free within ~1.5us; rope rotations follow on DVE
                        nc.vector.tensor_scalar_add(qT[:, 0, cs], psums[0], bq_t[:, 0:1])
                        nc.vector.tensor_scalar_add(qT[:, 1, cs], psums[1], bq_t[:, 1:2])
                        nc.scalar.activation(qT[:, 2, cs], psums[2], AF.Identity, bias=bq_t[:, 2:3])
                        nc.scalar.activation(qT[:, 3, cs], psums[3], AF.Identity, bias=bq_t[:, 3:4])
                        nc.scalar.activation(kT[:, cs], psums[4], AF.Identity, bias=bkv_t[:, 0:1])
                        nc.scalar.activation(vT[:, cs], psums[5], AF.Identity, bias=bkv_t[:, 1:2])
                        for m in range(NH_LOC):
                            _rope_rot(nc, ropetmp, qT[:, m, cs], cos_t[:, cs], sin_t[:, cs])
                        _rope_rot(nc, ropetmp, kT[:, cs], cos_t[:, cs], sin_t[:, cs])

                        # transpose this chunk's v: vT [hd, sk] -> vN [sk, hd]
                        for tl in range(CH // 128):
                            t = ch * (CH // 128) + tl
                            pst = tppsum.tile([128, 128], BF16)
                            nc.tensor.transpose(
                                pst[:, :], vT[:, t * 128 : (t + 1) * 128], ident_t[:, :]
                            )
                            nc.vector.tensor_copy(vN[:, t, :], pst[:, :])

                # phase B + C SBUF pools open together: the wo_t load streams
                # during attention, hiding the 8MB transfer
                with (
                    tc.tile_pool(name="wo", bufs=1) as wop,
                    tc.tile_pool(name="strip", bufs=3) as stripp,
                    tc.tile_pool(name="outp", bufs=3) as outp,
                ):
                    wo_tile = wop.tile([128, ND, QD], BF16)
                    for d in range(ND):
                        nc.sync.dma_start(
                            out=wo_tile[:, d, :],
                            in_=wo_t[d * 128 : (d + 1) * 128, :],
                        )

                    # ---------------- phase B: attention ---------------------
                    with (
                        tc.tile_pool(name="sppair", bufs=2, space="PSUM") as sppair,
                        tc.tile_pool(name="spdiag", bufs=2, space="PSUM") as spdiag,
                        tc.tile_pool(name="opsum", bufs=2, space="PSUM") as opsum,
                        tc.tile_pool(name="ptile", bufs=3) as ptile,
                        tc.tile_pool(name="accp", bufs=2) as accp,
                        tc.tile_pool(name="btmp", bufs=2) as btmp,
                    ):
                        ch_order = B_ORDER if causal else list(range(NCH))
                        for ch in ch_order:
                            cs = slice(ch * CH, (ch + 1) * CH)
                            nt = 4 * (ch + 1) if causal else NT
                            n_full = nt - 4 if causal else NT
                            for h in range(NH_LOC):
                                ps_o = opsum.tile([128, CH], F32, tag="o")
                                acc = accp.tile([128, CH], F32R, tag="acc")
                                first = True
                                # full tiles, processed in pairs sharing one
                                # 2-bank psum tile -> single exp instruction
                                for pr in range(n_full // 2):
                                    t0, t1 = 2 * pr, 2 * pr + 1
                                    ps2 = sppair.tile([128, 2, CH], F32, tag="sc2")
                                    for ti, t in enumerate((t0, t1)):
                                        nc.tensor.matmul(
                                            ps2[:, ti, :],
                                            kT[:, t * 128 : (t + 1) * 128],
                                            qT[:, h, cs],
                                            start=True,
                                            stop=True,
                                        )
                                    p2 = ptile.tile([128, 2, CH], BF16, tag="p2")
                                    if causal:
                                        nc.scalar.activation(
                                            p2[:, :, :], ps2[:, :, :], AF.Exp, scale=SCALE
                                        )
                                    else:
                                        for ti, t in enumerate((t0, t1)):
                                            mt = btmp.tile([128, CH], F32, tag="mt")
                                            nc.sync.dma_start(
                                                out=mt,
                                                in_=maskT[t * 128 : (t + 1) * 128, cs],
                                            )
                                            nc.vector.scalar_tensor_tensor(
                                                ps2[:, ti, :], ps2[:, ti, :], 1.0,
                                                mt[:, :], op0=ALU.mult, op1=ALU.add,
                                            )
                                        nc.scalar.activation(
                                            p2[:, :, :], ps2[:, :, :], AF.Exp, scale=SCALE
                                        )
                                    nc.tensor.matmul(
                                        ps_o[:, :], vN[:, t0, :], p2[:, 0, :],
                                        start=first, stop=False, skip_group_check=True,
                                    )
                                    nc.tensor.matmul(
                                        ps_o[:, :], vN[:, t1, :], p2[:, 1, :],
                                        start=False, stop=(not causal and pr == n_full // 2 - 1),
                                        skip_group_check=True,
                                    )
                                    # Pool-side softmax denominator accumulation
                                    if first:
                                        nc.gpsimd.tensor_add(
                                            acc[:, :], p2[:, 0, :], p2[:, 1, :]
                                        )
                                        first = False
                                    else:
                                        nc.gpsimd.tensor_add(acc[:, :], acc[:, :], p2[:, 0, :])
                                        nc.gpsimd.tensor_add(acc[:, :], acc[:, :], p2[:, 1, :])
                                # diagonal tiles (causal only)
                                if causal:
                                    for j in range(4):
                                        t = n_full + j
                                        sq0 = 128 * j  # valid-data start
                                        sq0m = min(sq0, 256)  # f32r >=256 matmul start
                                        # chunk 0 has no pair tiles: borrow the
                                        # idle pair-pool buffers (same tag ->
                                        # same banks) for a 4-deep diag pipeline
                                        if n_full == 0 and j % 2 == 1:
                                            psd = sppair.tile([128, CH], F32, tag="sc2")
                                        else:
                                            psd = spdiag.tile([128, CH], F32, tag="scd")
                                        nc.tensor.matmul(
                                            psd[:, sq0m:],
                                            kT[:, t * 128 : (t + 1) * 128],
                                            qT[:, h, ch * CH + sq0m : (ch + 1) * CH],
                                            start=True,
                                            stop=True,
                                        )
                                        # mask only the 128-wide diagonal block,
                                        # in place (mask pre-divided by SCALE)
                                        nc.vector.scalar_tensor_tensor(
                                            psd[:, sq0 : sq0 + 128],
                                            psd[:, sq0 : sq0 + 128],
                                            1.0,
                                            mask_t[:, j, sq0 : sq0 + 128],
                                            op0=ALU.mult,
                                            op1=ALU.add,
                                        )
                                        pd = ptile.tile([128, CH], BF16, tag="pd")
                                        nc.scalar.activation(
                                            pd[:, sq0:], psd[:, sq0:], AF.Exp, scale=SCALE
                                        )
                                        nc.tensor.matmul(
                                            ps_o[:, sq0:], vN[:, t, :], pd[:, sq0:],
                                            start=first, stop=(j == 3),
                                            skip_group_check=True,
                                        )
                                        if first:
                                            nc.gpsimd.tensor_copy(acc[:, :], pd[:, :])
                                            first = False
                                        else:
                                            nc.gpsimd.tensor_add(
                                                acc[:, sq0:], acc[:, sq0:], pd[:, sq0:]
                                            )
                                # normalize: single short ones-matmul reduces
                                # the Pool-accumulated acc across partitions;
                                # reciprocal + PE broadcast + multiply.  The
                                # sums/bcast psums ride the diag pool's
                                # buffer rotation (no extra banks).
                                spool, stag = spdiag, "scd"
                                sums = spool.tile([1, CH], F32, tag=stag, name="sums")
                                nc.tensor.matmul(
                                    sums[:, :], ones_col_t[:, :], acc[:, :],
                                    start=True, stop=True,
                                )
                                rec = accp.tile([1, CH], F32R, tag="rec")
                                with nc.allow_low_precision(reason="f32r recip"):
                                    nc.vector.reciprocal(rec[:, :], sums[:, :])
                                ps_bc = spool.tile([128, CH], F32, tag=stag, name="bcp")
                                nc.tensor.matmul(
                                    ps_bc[:, :], ones_row_t[:, :], rec[:, :],
                                    start=True, stop=True,
                                )
                                bc_sb = btmp.tile([128, CH], F32R, tag="bcs")
                                nc.vector.tensor_copy(bc_sb[:, :], ps_bc[:, :])
                                an = btmp.tile([128, CH], BF16, tag="an")
                                nc.vector.tensor_mul(an[:, :], ps_o[:, :], bc_sb[:, :])
                                nc.sync.dma_start(
                                    out=ag_in[ch, h * 128 : (h + 1) * 128, :],
                                    in_=an[:, :],
                                )
                            # fire this sq-chunk's AllGather as soon as its 4
                            # heads are done; overlaps remaining attention +
                            # output projection
                            if not skip_collective:
                                nc.gpsimd.collective_compute(
                                    "AllGather",
                                    ALU.bypass,
                                    ins=[ag_in[ch, :, :]],
                                    outs=[ag_out[ch, :, :]],
                                    replica_groups=[list(range(N_CORES))],
                                )

                    # ---------------- phase C: output projection ----------
                    with tc.tile_pool(name="copsum", bufs=2, space="PSUM") as copsum:
                        ps_bo = copsum.tile([128, QD], F32, tag="co0", name="psbo")
                        nc.tensor.matmul(
                            ps_bo[:, :], ones_row_t[:, :], bo_t[:, :],
                            start=True, stop=True,
                        )
                        bo_bc = outp.tile([128, QD], F32, tag="bo")
                        nc.vector.tensor_copy(bo_bc[:, :], ps_bo[:, :])

                        sqb_order = B_ORDER if causal else list(range(NCH))
                        for sqb in sqb_order:
                            ps_outs = [
                                copsum.tile(
                                    [128, QD], F32, tag=f"co{j}", name=f"co{j}"
                                )
                                for j in range(4)
                            ]
                            SG = 4  # strips per batched DMA
                            for dg in range(ND // SG):
                                d0 = dg * SG
                                strip = stripp.tile([128, SG, CH], BF16, tag="strip")
                                nc.sync.dma_start(
                                    out=strip,
                                    in_=ag_out[
                                        sqb, d0 * 128 : (d0 + SG) * 128, :
                                    ].rearrange("(g p) m -> p g m", p=128),
                                )
                                for dl in range(SG):
                                    d = d0 + dl
                                    for j in range(4):
                                        nc.tensor.matmul(
                                            ps_outs[j][:, :],
                                            strip[:, dl, j * 128 : (j + 1) * 128],
                                            wo_tile[:, d, :],
                                            start=(d == 0),
                                            stop=(d == ND - 1),
                                            skip_group_check=True,
                                        )
                            for j in range(4):
                                ot = outp.tile([128, QD], F32, tag="ot")
                                nc.vector.tensor_add(
                                    ot[:, :], ps_outs[j][:, :], bo_bc[:, :]
                                )
                                nc.sync.dma_start(
                                    out=out[
                                        sqb * CH + j * 128 : sqb * CH + (j + 1) * 128,
                                        :,
                                    ],
                                    in_=ot[:, :],
                                )

    if not skip_waitfix:
        _split_waits(nc)
    return nc


def _rope_rot(nc, tmp, dst, cos_s, sin_s):
    """In-place partial rope on dst[hd, s] (f32r): rows 0:64 rotated, rest pass.

    cos_s [64, CH]: cos table duplicated over both 32-row halves.
    sin_s [64, CH]: sign-folded sin: rows 0:32 = -sin, rows 32:64 = +sin, so
      new[0:64] = x[0:64]*cos_s + rot32(x[0:64])*sin_s
    with rot32 = swap of the two 32-row halves (done via SBUF->SBUF DMA,
    since compute engines cannot cross partitions).  All SBUF-only, so the
    muls/add run on the otherwise-idle Pool(GpSimd) engine, keeping DVE
    free for the attention phase that follows.
    """
    R2 = ROT // 2
    qsh = tmp.tile([ROT, CH], F32R, tag="r1")
    nc.sync.dma_start(out=qsh[0:R2, :], in_=dst[R2:ROT, :])
    nc.sync.dma_start(out=qsh[R2:ROT, :], in_=dst[0:R2, :])
    t1 = tmp.tile([ROT, CH], F32, tag="r2")
    nc.gpsimd.tensor_mul(t1[:, :], dst[0:ROT, :], cos_s[:, :])
    t2 = tmp.tile([ROT, CH], F32, tag="r3")
    nc.gpsimd.tensor_mul(t2[:, :], qsh[:, :], sin_s[:, :])
    nc.gpsimd.tensor_add(dst[0:ROT, :], t1[:, :], t2[:, :])


# --------------------------------------------------------------------------
# host side: shard, run, gather
# --------------------------------------------------------------------------
_NC_CACHE = {}


def _get_nc(causal: bool) -> bass.Bass:
    if causal not in _NC_CACHE:
        _NC_CACHE[causal] = build(causal)
    return _NC_CACHE[causal]


def _rope_tables():
    inv_freq = 1.0 / (BASE ** (np.arange(0, ROT, 2, dtype=np.float64) / ROT))
    t = np.arange(S, dtype=np.float64)
    freqs = np.outer(t, inv_freq)  # [S, 32]
    import ml_dtypes

    cos32 = np.cos(freqs).T.astype(np.float32)  # [32, S]
    sin32 = np.sin(freqs).T.astype(np.float32)
    cosT = np.concatenate([cos32, cos32], axis=0).astype(ml_dtypes.bfloat16)
    sinT = np.concatenate([-sin32, sin32], axis=0).astype(ml_dtypes.bfloat16)
    return cosT, sinT


def _check_causal(mask):
    """mask: [1,1,S,S]. True if it is exactly a causal additive mask."""
    m = mask[0, 0]
    if not (m[np.tril_indices(S)] == 0.0).all():
        return False
    iu = np.triu_indices(S, k=1)
    vals = m[iu]
    return bool((vals <= -1e30).all()) and bool((vals == vals[0]).all())


def _prescale_mask(m64):
    """Divide mask by SCALE in f64, clipped to the f32 range."""
    fmax = np.finfo(np.float32).max
    return np.clip(m64.astype(np.float64) / SCALE, -fmax, fmax).astype(np.float32)


def _make_in_maps(inputs, causal):
    import ml_dtypes
    hidden = np.asarray(inputs["hidden_states"], dtype=np.float32)
    mask = np.asarray(inputs["attention_mask"], dtype=np.float32)
    Wq = np.asarray(inputs["Wq"], dtype=np.float32)
    bq = np.asarray(inputs["bq"], dtype=np.float32)
    Wkv = np.asarray(inputs["Wkv"], dtype=np.float32)
    bkv = np.asarray(inputs["bkv"], dtype=np.float32)
    Wo = np.asarray(inputs["Wo"], dtype=np.float32)
    bo = np.asarray(inputs["bo"], dtype=np.float32)

    hidT_bf = np.ascontiguousarray(hidden[0].T).astype(ml_dtypes.bfloat16)  # [H, S]
    cosT, sinT = _rope_tables()
    ident = np.eye(128, dtype=ml_dtypes.bfloat16)
    ones_col = np.ones((128, 1), np.float32)
    ones_row = np.ones((1, 128), np.float32)

    if causal:
        # diagonal strips from the actual mask (chunk 0 is representative --
        # _check_causal guarantees the pattern is uniform along the diagonal)
        maskT = np.stack(
            [_prescale_mask(
                np.ascontiguousarray(mask[0, 0, 0:CH, 128 * j : 128 * j + 128].T))
             for j in range(4)]
        )  # [4, 128, CH]
    else:
        maskT = _prescale_mask(np.ascontiguousarray(mask[0, 0].T))  # [S, S]

    in_maps = []
    for c in range(N_CORES):
        qs = slice(c * QD, (c + 1) * QD)
        kvs = slice(c * KVD, (c + 1) * KVD)
        in_maps.append(
            {
                "hidT": hidT_bf,
                "wq_t": np.ascontiguousarray(Wq[qs, :].T).astype(ml_dtypes.bfloat16),
                "wkv_t": np.ascontiguousarray(Wkv[kvs, :].T).astype(ml_dtypes.bfloat16),
                "wo_t": np.ascontiguousarray(Wo[qs, :].T).astype(ml_dtypes.bfloat16),
                "bq": np.ascontiguousarray(bq[qs].reshape(NH_LOC, 128).T),
                "bkv": np.ascontiguousarray(
                    bkv[kvs].reshape(2, 128).T
                ),
                "bo": bo[qs].reshape(1, QD),
                "cosT": cosT,
                "sinT": sinT,
                "ident": ident,
                "ones_col": ones_col,
                "ones_row": ones_row,
                "maskT": maskT,
            }
        )
    return in_maps


def kernel(**inputs) -> np.ndarray:
    causal = _check_causal(np.asarray(inputs["attention_mask"], dtype=np.float32))
    nc = _get_nc(causal)
    in_maps = _make_in_maps(inputs, causal)
    res = run_bass_kernel_spmd(nc, in_maps, list(range(N_CORES)))
    outs = [res.results[c]["out"] for c in range(N_CORES)]  # each [S, QD]
    full = np.concatenate(outs, axis=1)  # [S, H]
    return full.reshape(B, S, H)


# revision 39
# speedup vs baseline: 1.0420x; 1.0024x over previous
"""Tensor-parallel GQA attention (CustomLlamaAttention) on 8 TRN2 NeuronCores.

Sharding: heads.  Core c owns Q heads 4c..4c+3 and KV head c.
  - Wq/Wkv output dims sharded; attention fully head-local per core.
  - Output projection sharded over Wo *rows* (output dim): each core computes
    out[:, 512c:512c+512] after an AllGather of the per-core attention
    outputs (transposed layout [hd, s]) -- cheaper than the all-reduce
    variant (4MB gather vs 32MB reduce).

Per-core dataflow (f32r q/k scores at full PE rate, bf16 p/v):
  hidT [4096,2048] -> Q/KV projections (d-major matmul order so the first
  matmuls only wait on 3 DMAs) -> qT/kT [hd,s] + partial RoPE (bias copies
  split across DVE/ACT/Pool so the PSUM banks free fast at chunk
  boundaries); vT -> PE-transpose -> vN [sk,hd] bf16.

  Attention per (chunk, head), [sk,sq] layout: paired score tiles share one
  2-bank PSUM tile so a single ACT exp covers both (halves ACT per-instr
  overhead); diagonal tiles get the additive mask (pre-divided by 1/sqrt(d)
  host-side) applied only to the 128-wide diagonal block, in place in PSUM.
  Softmax denominators NEVER touch the PE: the Pool(GpSimd) engine
  accumulates exp tiles (acc += p) and a partition_all_reduce replicates
  column sums to all partitions; DVE reciprocal+multiply normalizes.

  AllGather is chunked along sq (4 collectives), each fired as soon as its
  chunk's 4 heads are normalized, hiding collective latency behind the
  remaining attention chunks and the output projection.

  Phase order: projections [3,0,1,2], attention [2,3,0,1] so the first
  attention chunk has 8 unmasked score tiles of runway while the last
  projection chunk's rope stores drain on DVE.

Matmul/DMA instructions can carry only one semaphore wait on this
toolchain (single EVENTS slot in the ISA); waitfix splits excess waits
onto sequencer NOPs.
"""

import sys

sys.path.insert(0, "/opt/trn_rl_repo")

import numpy as np

import concourse.bass as bass
import concourse.mybir as mybir
import concourse.tile as tile
from concourse import bass_isa
from concourse.bass_utils import run_bass_kernel_spmd

# ---- problem constants (hardcoded per contract) ----
B, S, H = 1, 2048, 4096
NH, NKV, HD = 32, 8, 128
ROT = 64
BASE = 10000.0
N_CORES = 8
NH_LOC = NH // N_CORES  # 4 q heads per core
QD = NH_LOC * HD  # 512 local q dims
KVD = 2 * HD  # 256 local kv dims
CH = 512  # seq chunk (psum bank width in f32)
NCH = S // CH  # 4
ND = H // 128  # 32 contraction tiles
NT = S // 128  # 16 sk tiles
SCALE = 1.0 / float(np.sqrt(HD))

# natural chunk order both phases: attention chunk 0 only needs data from
# projection chunk 0 (done first), so phase B starts with zero stall
A_ORDER = [0, 1, 2, 3]
B_ORDER = [0, 1, 2, 3]

F32 = mybir.dt.float32
F32R = mybir.dt.float32r
BF16 = mybir.dt.bfloat16
AF = mybir.ActivationFunctionType
ALU = mybir.AluOpType


# --------------------------------------------------------------------------
# waitfix: split >1 semaphore waits per instruction onto sequencer NOPs
# --------------------------------------------------------------------------
def _split_waits(nc, max_waits=1):
    isa = nc.isa
    op = isa.Opcode.NEURON_ISA_TPB_OPCODE_NOP
    n_fixed = 0
    for f in nc.m.functions:
        for blk in f.blocks:
            il = blk.instructions
            fixes = []
            for i, inst in enumerate(il):
                si = inst.sync_info
                if si is None or len(si.on_wait) <= max_waits:
                    continue
                fixes.append((i, inst))
            for i, inst in reversed(fixes):
                si = inst.sync_info
                waits = list(si.on_wait)
                keep = waits[-max_waits:]
                nops = []
                for w in waits[:-max_waits]:
                    instr, fixups = bass_isa.isa_struct(isa, op, {})
                    nop = mybir.InstISA(
                        name=nc.get_next_instruction_name(),
                        isa_opcode=op.value,
                        engine=inst.engine,
                        instr=instr,
                        op_name="NOP",
                        ins=[],
                        outs=[],
                        ant_dict={},
                        verify=True,
                        ant_isa_is_sequencer_only=True,
                        ant_sbuf_fixups=fixups or None,
                    )
                    nop.sync_info = mybir.SyncInfo(on_wait=[w], on_update=[])
                    nops.append(nop)
                inst.sync_info = mybir.SyncInfo(on_wait=keep, on_update=si.on_update)
                for j, nop in enumerate(nops):
                    il.insert(i + j, nop)
                n_fixed += 1
    return n_fixed


# --------------------------------------------------------------------------
# kernel builder (SPMD program, same for all 8 cores)
# --------------------------------------------------------------------------
def build(causal: bool, skip_collective: bool = False, skip_waitfix: bool = False) -> bass.Bass:
    nc = bass.Bass()

    hidT = nc.declare_dram_parameter("hidT", [H, S], BF16, isOutput=False)
    wq_t = nc.declare_dram_parameter("wq_t", [H, QD], BF16, isOutput=False)
    wkv_t = nc.declare_dram_parameter("wkv_t", [H, KVD], BF16, isOutput=False)
    wo_t = nc.declare_dram_parameter("wo_t", [H, QD], BF16, isOutput=False)
    bq = nc.declare_dram_parameter("bq", [128, NH_LOC], F32, isOutput=False)
    bkv = nc.declare_dram_parameter("bkv", [128, 2], F32, isOutput=False)
    bo = nc.declare_dram_parameter("bo", [1, QD], F32R, isOutput=False)
    ones_col = nc.declare_dram_parameter("ones_col", [128, 1], F32R, isOutput=False)
    ones_row = nc.declare_dram_parameter("ones_row", [1, 128], F32R, isOutput=False)
    cosT = nc.declare_dram_parameter("cosT", [ROT, S], BF16, isOutput=False)
    sinT = nc.declare_dram_parameter("sinT", [ROT, S], BF16, isOutput=False)
    ident = nc.declare_dram_parameter("ident", [128, 128], BF16, isOutput=False)
    if causal:
        # 4 diagonal mask strips: strip j is the [sk 128, sq 512] transposed
        # mask block whose diagonal offset is 128*j.  Values pre-divided by
        # SCALE (clipped to f32 range) so exp's scale multiply re-applies it.
        maskT = nc.declare_dram_parameter("maskT", [4, 128, CH], F32, isOutput=False)
    else:
        maskT = nc.declare_dram_parameter("maskT", [S, S], F32, isOutput=False)
    out = nc.declare_dram_parameter("out", [S, QD], F32, isOutput=True)

    # chunk-major layout so each sq-chunk's collective operates on a
    # contiguous DRAM region
    ag_in = nc.dram_tensor("ag_in", [NCH, QD, CH], BF16)
    ag_out = nc.dram_tensor("ag_out", [NCH, H, CH], BF16, addr_space="Shared")

    with tile.TileContext(nc) as tc:
        with tc.tile_pool(name="consts", bufs=1) as consts:
            # all const tables are DMA'd after chunk 0's first d-group so
            # they don't delay the first matmuls
            bq_t = consts.tile([128, NH_LOC], F32)
            bkv_t = consts.tile([128, 2], F32)
            ident_t = consts.tile([128, 128], BF16)
            cos_t = consts.tile([ROT, S], BF16)
            sin_t = consts.tile([ROT, S], BF16)
            bo_t = consts.tile([1, QD], F32R)
            ones_col_t = consts.tile([128, 1], F32R)
            ones_row_t = consts.tile([1, 128], F32R)
            mask_t = consts.tile([128, 4, CH], F32, name="mask_t") if causal else None

            def _load_big_consts():
                nc.sync.dma_start(out=ones_col_t, in_=ones_col[:, :])
                nc.sync.dma_start(out=ones_row_t, in_=ones_row[:, :])
                nc.sync.dma_start(out=bq_t, in_=bq[:, :])
                nc.sync.dma_start(out=bkv_t, in_=bkv[:, :])
                nc.sync.dma_start(out=cos_t, in_=cosT[:, :])
                nc.sync.dma_start(out=sin_t, in_=sinT[:, :])
                nc.sync.dma_start(out=ident_t, in_=ident[:, :])
                nc.sync.dma_start(out=bo_t, in_=bo[:, :])
                if causal:
                    nc.sync.dma_start(
                        out=mask_t, in_=maskT.rearrange("j p m -> p j m")
                    )

            with tc.tile_pool(name="qkv", bufs=1) as qkv:
                # per-chunk tiles: dependency tracking is per-tile, so the
                # attention phase's chunk-0 reads only wait on projection
                # chunk 0, not on the last chunk's rope chain
                qT_ch = [qkv.tile([128, NH_LOC, CH], F32R, name=f"qT{c}") for c in range(NCH)]
                kT_ch = [qkv.tile([128, CH], F32R, name=f"kT{c}") for c in range(NCH)]
                vN_ch = [qkv.tile([128, CH // 128, HD], BF16, name=f"vN{c}") for c in range(NCH)]

                # ---------------- phase A: projections -------------------
                with (
                    tc.tile_pool(name="wqkv", bufs=1) as wpool,
                    tc.tile_pool(name="hid", bufs=3) as hidp,
                    tc.tile_pool(name="ppsum", bufs=1, space="PSUM") as ppsum,
                    tc.tile_pool(name="ropetmp", bufs=2) as ropetmp,
                    tc.tile_pool(name="vtmp", bufs=1) as vtmp,
                    tc.tile_pool(name="tppsum", bufs=2, space="PSUM") as tppsum,
                ):
                    # weight tiles are loaded per-d, interleaved with the first
                    # chunk's hid tiles so the first matmuls start after ~3
                    # small DMAs instead of the full 12MB
                    wq_tile = wpool.tile([128, ND, QD], BF16)
                    wkv_tile = wpool.tile([128, ND, KVD], BF16)
                    vT = vtmp.tile([128, S], BF16)  # [hd, sk], pre-transpose

                    DG = 8
                    for ch_i, ch in enumerate(A_ORDER):
                        cs = slice(ch * CH, (ch + 1) * CH)
                        psums = [ppsum.tile([128, CH], F32, tag=f"pp{m}", name=f"pp{m}") for m in range(6)]
                        for grp in range(ND // DG):
                            if ch_i == 0 and grp == 1:
                                _load_big_consts()
                            d0 = grp * DG
                            # batched DMAs: 2 hid half-group loads + 1 per
                            # weight tensor per group (SP issue rate is the
                            # DMA bottleneck, not bandwidth)
                            ht_g = hidp.tile([128, DG, CH], BF16, tag="hid")
                            for half in range(2):
                                hd0 = d0 + half * (DG // 2)
                                nc.sync.dma_start(
                                    out=ht_g[:, half * (DG // 2) : (half + 1) * (DG // 2), :],
                                    in_=hidT[
                                        hd0 * 128 : (hd0 + DG // 2) * 128, cs
                                    ].rearrange("(g p) m -> p g m", p=128),
                                )
                            if ch_i == 0:
                                for half in range(2):
                                    hd0 = d0 + half * (DG // 2)
                                    nc.sync.dma_start(
                                        out=wq_tile[:, hd0 : hd0 + DG // 2, :],
                                        in_=wq_t[
                                            hd0 * 128 : (hd0 + DG // 2) * 128, :
                                        ].rearrange("(g p) m -> p g m", p=128),
                                    )
                                    nc.sync.dma_start(
                                        out=wkv_tile[:, hd0 : hd0 + DG // 2, :],
                                        in_=wkv_t[
                                            hd0 * 128 : (hd0 + DG // 2) * 128, :
                                        ].rearrange("(g p) m -> p g m", p=128),
                                    )
                            # d-major: the first 6 matmuls need only the first
                            # half-group hid DMA + this group's weight DMAs
                            for dl in range(DG):
                                d = d0 + dl
                                for m in range(6):
                                    if m < NH_LOC:
                                        w_ap = wq_tile[:, :, m * 128 : (m + 1) * 128]
                                    else:
                                        mm = m - NH_LOC
                                        w_ap = wkv_tile[:, :, mm * 128 : (mm + 1) * 128]
                                    nc.tensor.matmul(
                                        psums[m][:, :],
                                        w_ap[:, d, :],
                                        ht_g[:, dl, :],
                                        start=(d == 0),
                                        stop=(d == ND - 1),
                                        skip_group_check=True,
                                    )
                        # bias copies split across DVE/ACT so all 6 psums
                        # free within ~1.5us; rope rotations follow on Pool
                        nc.vector.tensor_scalar_add(qT_ch[ch][:, 0, :], psums[0], bq_t[:, 0:1])
                        nc.vector.tensor_scalar_add(qT_ch[ch][:, 1, :], psums[1], bq_t[:, 1:2])
                        nc.scalar.activation(qT_ch[ch][:, 2, :], psums[2], AF.Identity, bias=bq_t[:, 2:3])
                        nc.scalar.activation(qT_ch[ch][:, 3, :], psums[3], AF.Identity, bias=bq_t[:, 3:4])
                        nc.scalar.activation(kT_ch[ch][:, :], psums[4], AF.Identity, bias=bkv_t[:, 0:1])
                        nc.scalar.activation(vT[:, cs], psums[5], AF.Identity, bias=bkv_t[:, 1:2])
                        for m in range(NH_LOC):
                            _rope_rot(nc, ropetmp, qT_ch[ch][:, m, :], cos_t[:, cs], sin_t[:, cs])
                        _rope_rot(nc, ropetmp, kT_ch[ch][:, :], cos_t[:, cs], sin_t[:, cs])

                        # transpose this chunk's v: vT [hd, sk] -> vN [sk, hd]
                        for tl in range(CH // 128):
                            t = ch * (CH // 128) + tl
                            pst = tppsum.tile([128, 128], BF16)
                            nc.tensor.transpose(
                                pst[:, :], vT[:, t * 128 : (t + 1) * 128], ident_t[:, :]
                            )
                            nc.vector.tensor_copy(vN_ch[ch][:, tl, :], pst[:, :])

                # phase B + C SBUF pools open together: the wo_t load streams
                # during attention, hiding the 8MB transfer
                with (
                    tc.tile_pool(name="wo", bufs=1) as wop,
                    tc.tile_pool(name="strip", bufs=3) as stripp,
                    tc.tile_pool(name="outp", bufs=3) as outp,
                ):
                    wo_tile = wop.tile([128, ND, QD], BF16)
                    for d in range(ND):
                        nc.sync.dma_start(
                            out=wo_tile[:, d, :],
                            in_=wo_t[d * 128 : (d + 1) * 128, :],
                        )

                    # ---------------- phase B: attention ---------------------
                    with (
                        tc.tile_pool(name="sppair", bufs=2, space="PSUM") as sppair,
                        tc.tile_pool(name="spdiag", bufs=2, space="PSUM") as spdiag,
                        tc.tile_pool(name="opsum", bufs=2, space="PSUM") as opsum,
                        tc.tile_pool(name="ptile", bufs=3) as ptile,
                        tc.tile_pool(name="accp", bufs=2) as accp,
                        tc.tile_pool(name="btmp", bufs=2) as btmp,
                    ):
                        ch_order = B_ORDER if causal else list(range(NCH))
                        for ch in ch_order:
                            cs = slice(ch * CH, (ch + 1) * CH)
                            nt = 4 * (ch + 1) if causal else NT
                            n_full = nt - 4 if causal else NT
                            for h in range(NH_LOC):
                                ps_o = opsum.tile([128, CH], F32, tag="o")
                                acc = accp.tile([128, CH], F32R, tag="acc")
                                first = True
                                # full tiles, processed in pairs sharing one
                                # 2-bank psum tile -> single exp instruction
                                for pr in range(n_full // 2):
                                    t0, t1 = 2 * pr, 2 * pr + 1
                                    ps2 = sppair.tile([128, 2, CH], F32, tag="sc2")
                                    for ti, t in enumerate((t0, t1)):
                                        nc.tensor.matmul(
                                            ps2[:, ti, :],
                                            kT_ch[t // 4][:, (t % 4) * 128 : (t % 4 + 1) * 128],
                                            qT_ch[ch][:, h, :],
                                            start=True,
                                            stop=True,
                                        )
                                    p2 = ptile.tile([128, 2, CH], BF16, tag="p2")
                                    if causal:
                                        nc.scalar.activation(
                                            p2[:, :, :], ps2[:, :, :], AF.Exp, scale=SCALE
                                        )
                                    else:
                                        for ti, t in enumerate((t0, t1)):
                                            mt = btmp.tile([128, CH], F32, tag="mt")
                                            nc.sync.dma_start(
                                                out=mt,
                                                in_=maskT[t * 128 : (t + 1) * 128, cs],
                                            )
                                            nc.vector.scalar_tensor_tensor(
                                                ps2[:, ti, :], ps2[:, ti, :], 1.0,
                                                mt[:, :], op0=ALU.mult, op1=ALU.add,
                                            )
                                        nc.scalar.activation(
                                            p2[:, :, :], ps2[:, :, :], AF.Exp, scale=SCALE
                                        )
                                    nc.tensor.matmul(
                                        ps_o[:, :], vN_ch[t0 // 4][:, t0 % 4, :], p2[:, 0, :],
                                        start=first, stop=False, skip_group_check=True,
                                    )
                                    nc.tensor.matmul(
                                        ps_o[:, :], vN_ch[t1 // 4][:, t1 % 4, :], p2[:, 1, :],
                                        start=False, stop=(not causal and pr == n_full // 2 - 1),
                                        skip_group_check=True,
                                    )
                                    # Pool-side softmax denominator accumulation
                                    if first:
                                        nc.gpsimd.tensor_add(
                                            acc[:, :], p2[:, 0, :], p2[:, 1, :]
                                        )
                                        first = False
                                    else:
                                        nc.gpsimd.tensor_add(acc[:, :], acc[:, :], p2[:, 0, :])
                                        nc.gpsimd.tensor_add(acc[:, :], acc[:, :], p2[:, 1, :])
                                # diagonal tiles (causal only)
                                if causal:
                                    for j in range(4):
                                        t = n_full + j
                                        sq0 = 128 * j  # valid-data start
                                        sq0m = min(sq0, 256)  # f32r >=256 matmul start
                                        # chunk 0 has no pair tiles: borrow the
                                        # idle pair-pool buffers (same tag ->
                                        # same banks) for a 4-deep diag pipeline
                                        if n_full == 0 and j % 2 == 1:
                                            psd = sppair.tile([128, CH], F32, tag="sc2")
                                        else:
                                            psd = spdiag.tile([128, CH], F32, tag="scd")
                                        nc.tensor.matmul(
                                            psd[:, sq0m:],
                                            kT_ch[t // 4][:, (t % 4) * 128 : (t % 4 + 1) * 128],
                                            qT_ch[ch][:, h, sq0m:],
                                            start=True,
                                            stop=True,
                                        )
                                        # mask only the 128-wide diagonal block,
                                        # in place (mask pre-divided by SCALE)
                                        nc.vector.scalar_tensor_tensor(
                                            psd[:, sq0 : sq0 + 128],
                                            psd[:, sq0 : sq0 + 128],
                                            1.0,
                                            mask_t[:, j, sq0 : sq0 + 128],
                                            op0=ALU.mult,
                                            op1=ALU.add,
                                        )
                                        pd = ptile.tile([128, CH], BF16, tag="pd")
                                        nc.scalar.activation(
                                            pd[:, sq0:], psd[:, sq0:], AF.Exp, scale=SCALE
                                        )
                                        nc.tensor.matmul(
                                            ps_o[:, sq0:], vN_ch[t // 4][:, t % 4, :], pd[:, sq0:],
                                            start=first, stop=(j == 3),
                                            skip_group_check=True,
                                        )
                                        if first:
                                            nc.gpsimd.tensor_copy(acc[:, :], pd[:, :])
                                            first = False
                                        else:
                                            nc.gpsimd.tensor_add(
                                                acc[:, sq0:], acc[:, sq0:], pd[:, sq0:]
                                            )
                                # normalize: single short ones-matmul reduces
                                # the Pool-accumulated acc across partitions;
                                # reciprocal + PE broadcast + multiply.  The
                                # sums/bcast psums ride the diag pool's
                                # buffer rotation (no extra banks).
                                spool, stag = spdiag, "scd"
                                sums = spool.tile([1, CH], F32, tag=stag, name="sums")
                                nc.tensor.matmul(
                                    sums[:, :], ones_col_t[:, :], acc[:, :],
                                    start=True, stop=True,
                                )
                                rec = accp.tile([1, CH], F32R, tag="rec")
                                with nc.allow_low_precision(reason="f32r recip"):
                                    nc.vector.reciprocal(rec[:, :], sums[:, :])
                                ps_bc = spool.tile([128, CH], F32, tag=stag, name="bcp")
                                nc.tensor.matmul(
                                    ps_bc[:, :], ones_row_t[:, :], rec[:, :],
                                    start=True, stop=True,
                                )
                                bc_sb = btmp.tile([128, CH], F32R, tag="bcs")
                                nc.vector.tensor_copy(bc_sb[:, :], ps_bc[:, :])
                                an = btmp.tile([128, CH], BF16, tag="an")
                                nc.vector.tensor_mul(an[:, :], ps_o[:, :], bc_sb[:, :])
                                nc.sync.dma_start(
                                    out=ag_in[ch, h * 128 : (h + 1) * 128, :],
                                    in_=an[:, :],
                                )
                            # fire this sq-chunk's AllGather as soon as its 4
                            # heads are done; overlaps remaining attention +
                            # output projection
                            if not skip_collective:
                                nc.gpsimd.collective_compute(
                                    "AllGather",
                                    ALU.bypass,
                                    ins=[ag_in[ch, :, :]],
                                    outs=[ag_out[ch, :, :]],
                                    replica_groups=[list(range(N_CORES))],
                                )

                    # ---------------- phase C: output projection ----------
                    with tc.tile_pool(name="copsum", bufs=2, space="PSUM") as copsum:
                        ps_bo = copsum.tile([128, QD], F32, tag="co0", name="psbo")
                        nc.tensor.matmul(
                            ps_bo[:, :], ones_row_t[:, :], bo_t[:, :],
                            start=True, stop=True,
                        )
                        bo_bc = outp.tile([128, QD], F32, tag="bo")
                        nc.vector.tensor_copy(bo_bc[:, :], ps_bo[:, :])

                        sqb_order = B_ORDER if causal else list(range(NCH))
                        for sqb in sqb_order:
                            ps_outs = [
                                copsum.tile(
                                    [128, QD], F32, tag=f"co{j}", name=f"co{j}"
                                )
                                for j in range(4)
                            ]
                            SG = 4  # strips per batched DMA
                            for dg in range(ND // SG):
                                d0 = dg * SG
                                strip = stripp.tile([128, SG, CH], BF16, tag="strip")
                                nc.sync.dma_start(
                                    out=strip,
                                    in_=ag_out[
                                        sqb, d0 * 128 : (d0 + SG) * 128, :
                                    ].rearrange("(g p) m -> p g m", p=128),
                                )
                                for dl in range(SG):
                                    d = d0 + dl
                                    for j in range(4):
                                        nc.tensor.matmul(
                                            ps_outs[j][:, :],
                                            strip[:, dl, j * 128 : (j + 1) * 128],
                                            wo_tile[:, d, :],
                                            start=(d == 0),
                                            stop=(d == ND - 1),
                                            skip_group_check=True,
                                        )
                            for j in range(4):
                                ot = outp.tile([128, QD], F32, tag="ot")
                                nc.vector.tensor_add(
                                    ot[:, :], ps_outs[j][:, :], bo_bc[:, :]
                                )
                                nc.sync.dma_start(
                                    out=out[
                                        sqb * CH + j * 128 : sqb * CH + (j + 1) * 128,
                                        :,
                                    ],
                                    in_=ot[:, :],
                                )

    if not skip_waitfix:
        _split_waits(nc)
    return nc


def _rope_rot(nc, tmp, dst, cos_s, sin_s):
    """In-place partial rope on dst[hd, s] (f32r): rows 0:64 rotated, rest pass.

    cos_s [64, CH]: cos table duplicated over both 32-row halves.
    sin_s [64, CH]: sign-folded sin: rows 0:32 = -sin, rows 32:64 = +sin, so
      new[0:64] = x[0:64]*cos_s + rot32(x[0:64])*sin_s
    with rot32 = swap of the two 32-row halves (done via SBUF->SBUF DMA,
    since compute engines cannot cross partitions).  All SBUF-only, so the
    muls/add run on the otherwise-idle Pool(GpSimd) engine, keeping DVE
    free for the attention phase that follows.
    """
    R2 = ROT // 2
    qsh = tmp.tile([ROT, CH], F32R, tag="r1")
    nc.sync.dma_start(out=qsh[0:R2, :], in_=dst[R2:ROT, :])
    nc.sync.dma_start(out=qsh[R2:ROT, :], in_=dst[0:R2, :])
    t1 = tmp.tile([ROT, CH], F32, tag="r2")
    nc.gpsimd.tensor_mul(t1[:, :], dst[0:ROT, :], cos_s[:, :])
    t2 = tmp.tile([ROT, CH], F32, tag="r3")
    nc.gpsimd.tensor_mul(t2[:, :], qsh[:, :], sin_s[:, :])
    nc.gpsimd.tensor_add(dst[0:ROT, :], t1[:, :], t2[:, :])


# --------------------------------------------------------------------------
# host side: shard, run, gather
# --------------------------------------------------------------------------
_NC_CACHE = {}


def _get_nc(causal: bool) -> bass.Bass:
    if causal not in _NC_CACHE:
        _NC_CACHE[causal] = build(causal)
    return _NC_CACHE[causal]


def _rope_tables():
    inv_freq = 1.0 / (BASE ** (np.arange(0, ROT, 2, dtype=np.float64) / ROT))
    t = np.arange(S, dtype=np.float64)
    freqs = np.outer(t, inv_freq)  # [S, 32]
    import ml_dtypes

    cos32 = np.cos(freqs).T.astype(np.float32)  # [32, S]
    sin32 = np.sin(freqs).T.astype(np.float32)
    cosT = np.concatenate([cos32, cos32], axis=0).astype(ml_dtypes.bfloat16)
    sinT = np.concatenate([-sin32, sin32], axis=0).astype(ml_dtypes.bfloat16)
    return cosT, sinT


def _check_causal(mask):
    """mask: [1,1,S,S]. True if it is exactly a causal additive mask."""
    m = mask[0, 0]
    if not (m[np.tril_indices(S)] == 0.0).all():
        return False
    iu = np.triu_indices(S, k=1)
    vals = m[iu]
    return bool((vals <= -1e30).all()) and bool((vals == vals[0]).all())


def _prescale_mask(m64):
    """Divide mask by SCALE in f64, clipped to the f32 range."""
    fmax = np.finfo(np.float32).max
    return np.clip(m64.astype(np.float64) / SCALE, -fmax, fmax).astype(np.float32)


def _make_in_maps(inputs, causal):
    import ml_dtypes
    hidden = np.asarray(inputs["hidden_states"], dtype=np.float32)
    mask = np.asarray(inputs["attention_mask"], dtype=np.float32)
    Wq = np.asarray(inputs["Wq"], dtype=np.float32)
    bq = np.asarray(inputs["bq"], dtype=np.float32)
    Wkv = np.asarray(inputs["Wkv"], dtype=np.float32)
    bkv = np.asarray(inputs["bkv"], dtype=np.float32)
    Wo = np.asarray(inputs["Wo"], dtype=np.float32)
    bo = np.asarray(inputs["bo"], dtype=np.float32)

    hidT_bf = np.ascontiguousarray(hidden[0].T).astype(ml_dtypes.bfloat16)  # [H, S]
    cosT, sinT = _rope_tables()
    ident = np.eye(128, dtype=ml_dtypes.bfloat16)
    ones_col = np.ones((128, 1), np.float32)
    ones_row = np.ones((1, 128), np.float32)

    if causal:
        # diagonal strips from the actual mask (chunk 0 is representative --
        # _check_causal guarantees the pattern is uniform along the diagonal)
        maskT = np.stack(
            [_prescale_mask(
                np.ascontiguousarray(mask[0, 0, 0:CH, 128 * j : 128 * j + 128].T))
             for j in range(4)]
        )  # [4, 128, CH]
    else:
        maskT = _prescale_mask(np.ascontiguousarray(mask[0, 0].T))  # [S, S]

    in_maps = []
    for c in range(N_CORES):
        qs = slice(c * QD, (c + 1) * QD)
        kvs = slice(c * KVD, (c + 1) * KVD)
        in_maps.append(
            {
                "hidT": hidT_bf,
                "wq_t": np.ascontiguousarray(Wq[qs, :].T).astype(ml_dtypes.bfloat16),
                "wkv_t": np.ascontiguousarray(Wkv[kvs, :].T).astype(ml_dtypes.bfloat16),
                "wo_t": np.ascontiguousarray(Wo[qs, :].T).astype(ml_dtypes.bfloat16),
                "bq": np.ascontiguousarray(bq[qs].reshape(NH_LOC, 128).T),
                "bkv": np.ascontiguousarray(
                    bkv[kvs].reshape(2, 128).T
                ),
                "bo": bo[qs].reshape(1, QD),
                "cosT": cosT,
                "sinT": sinT,
                "ident": ident,
                "ones_col": ones_col,
                "ones_row": ones_row,
                "maskT": maskT,
            }
        )
    return in_maps


def kernel(**inputs) -> np.ndarray:
    causal = _check_causal(np.asarray(inputs["attention_mask"], dtype=np.float32))
    nc = _get_nc(causal)
    in_maps = _make_in_maps(inputs, causal)
    res = run_bass_kernel_spmd(nc, in_maps, list(range(N_CORES)))
    outs = [res.results[c]["out"] for c in range(N_CORES)]  # each [S, QD]
    full = np.concatenate(outs, axis=1)  # [S, H]
    return full.reshape(B, S, H)


# revision 44
# speedup vs baseline: 1.0501x; 1.0078x over previous
"""Tensor-parallel GQA attention (CustomLlamaAttention) on 8 TRN2 NeuronCores.

Sharding: heads.  Core c owns Q heads 4c..4c+3 and KV head c.
  - Wq/Wkv output dims sharded; attention fully head-local per core.
  - Output projection sharded over Wo *rows* (output dim): each core computes
    out[:, 512c:512c+512] after an AllGather of the per-core attention
    outputs (transposed layout [hd, s]) -- cheaper than the all-reduce
    variant (4MB gather vs 32MB reduce).

Per-core dataflow (f32r q/k scores at full PE rate, bf16 p/v):
  hidT [4096,2048] -> Q/KV projections (d-major matmul order so the first
  matmuls only wait on 3 DMAs) -> qT/kT [hd,s] + partial RoPE (bias copies
  split across DVE/ACT/Pool so the PSUM banks free fast at chunk
  boundaries); vT -> PE-transpose -> vN [sk,hd] bf16.

  Attention per (chunk, head), [sk,sq] layout: paired score tiles share one
  2-bank PSUM tile so a single ACT exp covers both (halves ACT per-instr
  overhead); diagonal tiles get the additive mask (pre-divided by 1/sqrt(d)
  host-side) applied only to the 128-wide diagonal block, in place in PSUM.
  Softmax denominators NEVER touch the PE: the Pool(GpSimd) engine
  accumulates exp tiles (acc += p) and a partition_all_reduce replicates
  column sums to all partitions; DVE reciprocal+multiply normalizes.

  AllGather is chunked along sq (4 collectives), each fired as soon as its
  chunk's 4 heads are normalized, hiding collective latency behind the
  remaining attention chunks and the output projection.

  Phase order: projections [3,0,1,2], attention [2,3,0,1] so the first
  attention chunk has 8 unmasked score tiles of runway while the last
  projection chunk's rope stores drain on DVE.

Matmul/DMA instructions can carry only one semaphore wait on this
toolchain (single EVENTS slot in the ISA); waitfix splits excess waits
onto sequencer NOPs.
"""

import sys

sys.path.insert(0, "/opt/trn_rl_repo")

import numpy as np

import concourse.bass as bass
import concourse.mybir as mybir
import concourse.tile as tile
from concourse import bass_isa
from concourse.bass_utils import run_bass_kernel_spmd

# ---- problem constants (hardcoded per contract) ----
B, S, H = 1, 2048, 4096
NH, NKV, HD = 32, 8, 128
ROT = 64
BASE = 10000.0
N_CORES = 8
NH_LOC = NH // N_CORES  # 4 q heads per core
QD = NH_LOC * HD  # 512 local q dims
KVD = 2 * HD  # 256 local kv dims
CH = 512  # seq chunk (psum bank width in f32)
NCH = S // CH  # 4
ND = H // 128  # 32 contraction tiles
NT = S // 128  # 16 sk tiles
SCALE = 1.0 / float(np.sqrt(HD))

# natural chunk order both phases: attention chunk 0 only needs data from
# projection chunk 0 (done first), so phase B starts with zero stall
A_ORDER = [0, 1, 2, 3]
B_ORDER = [0, 1, 2, 3]

F32 = mybir.dt.float32
F32R = mybir.dt.float32r
BF16 = mybir.dt.bfloat16
AF = mybir.ActivationFunctionType
ALU = mybir.AluOpType


# --------------------------------------------------------------------------
# waitfix: split >1 semaphore waits per instruction onto sequencer NOPs
# --------------------------------------------------------------------------
def _split_waits(nc, max_waits=1):
    isa = nc.isa
    op = isa.Opcode.NEURON_ISA_TPB_OPCODE_NOP
    n_fixed = 0
    for f in nc.m.functions:
        for blk in f.blocks:
            il = blk.instructions
            fixes = []
            for i, inst in enumerate(il):
                si = inst.sync_info
                if si is None or len(si.on_wait) <= max_waits:
                    continue
                fixes.append((i, inst))
            for i, inst in reversed(fixes):
                si = inst.sync_info
                waits = list(si.on_wait)
                keep = waits[-max_waits:]
                nops = []
                for w in waits[:-max_waits]:
                    instr, fixups = bass_isa.isa_struct(isa, op, {})
                    nop = mybir.InstISA(
                        name=nc.get_next_instruction_name(),
                        isa_opcode=op.value,
                        engine=inst.engine,
                        instr=instr,
                        op_name="NOP",
                        ins=[],
                        outs=[],
                        ant_dict={},
                        verify=True,
                        ant_isa_is_sequencer_only=True,
                        ant_sbuf_fixups=fixups or None,
                    )
                    nop.sync_info = mybir.SyncInfo(on_wait=[w], on_update=[])
                    nops.append(nop)
                inst.sync_info = mybir.SyncInfo(on_wait=keep, on_update=si.on_update)
                for j, nop in enumerate(nops):
                    il.insert(i + j, nop)
                n_fixed += 1
    return n_fixed


# --------------------------------------------------------------------------
# kernel builder (SPMD program, same for all 8 cores)
# --------------------------------------------------------------------------
def build(causal: bool, skip_collective: bool = False, skip_waitfix: bool = False) -> bass.Bass:
    nc = bass.Bass()

    hidT = nc.declare_dram_parameter("hidT", [H, S], BF16, isOutput=False)
    wq_t = nc.declare_dram_parameter("wq_t", [H, QD], BF16, isOutput=False)
    wkv_t = nc.declare_dram_parameter("wkv_t", [H, KVD], BF16, isOutput=False)
    wo_t = nc.declare_dram_parameter("wo_t", [H, QD], BF16, isOutput=False)
    bq = nc.declare_dram_parameter("bq", [128, NH_LOC], F32, isOutput=False)
    bkv = nc.declare_dram_parameter("bkv", [128, 2], F32, isOutput=False)
    bo = nc.declare_dram_parameter("bo", [1, QD], F32R, isOutput=False)
    ones_col = nc.declare_dram_parameter("ones_col", [128, 1], F32R, isOutput=False)
    ones_row = nc.declare_dram_parameter("ones_row", [1, 128], F32R, isOutput=False)
    cosT = nc.declare_dram_parameter("cosT", [ROT, S], BF16, isOutput=False)
    sinT = nc.declare_dram_parameter("sinT", [ROT, S], BF16, isOutput=False)
    ident = nc.declare_dram_parameter("ident", [128, 128], BF16, isOutput=False)
    if causal:
        # 4 diagonal mask strips: strip j is the [sk 128, sq 512] transposed
        # mask block whose diagonal offset is 128*j.  Values pre-divided by
        # SCALE (clipped to f32 range) so exp's scale multiply re-applies it.
        maskT = nc.declare_dram_parameter("maskT", [4, 128, CH], F32, isOutput=False)
    else:
        maskT = nc.declare_dram_parameter("maskT", [S, S], F32, isOutput=False)
    out = nc.declare_dram_parameter("out", [S, QD], F32, isOutput=True)

    # chunk-major layout so each sq-chunk's collective operates on a
    # contiguous DRAM region
    ag_in = nc.dram_tensor("ag_in", [NCH, QD, CH], BF16)
    ag_out = nc.dram_tensor("ag_out", [NCH, H, CH], BF16, addr_space="Shared")

    with tile.TileContext(nc) as tc:
        with tc.tile_pool(name="consts", bufs=1) as consts:
            # all const tables are DMA'd after chunk 0's first d-group so
            # they don't delay the first matmuls
            bq_t = consts.tile([128, NH_LOC], F32)
            bkv_t = consts.tile([128, 2], F32)
            ident_t = consts.tile([128, 128], BF16)
            cos_t = consts.tile([ROT, S], BF16)
            sin_t = consts.tile([ROT, S], BF16)
            bo_t = consts.tile([1, QD], F32R)
            ones_col_t = consts.tile([128, 1], F32R)
            ones_row_t = consts.tile([1, 128], F32R)
            mask_t = consts.tile([128, 4, CH], F32, name="mask_t") if causal else None

            def _load_big_consts():
                nc.sync.dma_start(out=ones_col_t, in_=ones_col[:, :])
                nc.sync.dma_start(out=ones_row_t, in_=ones_row[:, :])
                nc.sync.dma_start(out=bq_t, in_=bq[:, :])
                nc.sync.dma_start(out=bkv_t, in_=bkv[:, :])
                nc.sync.dma_start(out=cos_t, in_=cosT[:, :])
                nc.sync.dma_start(out=sin_t, in_=sinT[:, :])
                nc.sync.dma_start(out=ident_t, in_=ident[:, :])
                nc.sync.dma_start(out=bo_t, in_=bo[:, :])
                if causal:
                    nc.sync.dma_start(
                        out=mask_t, in_=maskT.rearrange("j p m -> p j m")
                    )

            with (
                tc.tile_pool(name="qkv", bufs=1) as qkv,
                # rope scratch lives past phase A so phase-B pools don't
                # alias memory still being read by the rope chain's tail
                tc.tile_pool(name="ropetmp", bufs=2) as ropetmp,
            ):
                # per-chunk tiles: dependency tracking is per-tile, so the
                # attention phase's chunk-0 reads only wait on projection
                # chunk 0, not on the last chunk's rope chain
                qT_ch = [qkv.tile([128, NH_LOC, CH], F32R, name=f"qT{c}") for c in range(NCH)]
                kT_ch = [qkv.tile([128, CH], F32R, name=f"kT{c}") for c in range(NCH)]
                vN_ch = [qkv.tile([128, CH // 128, HD], BF16, name=f"vN{c}") for c in range(NCH)]

                # ---------------- phase A: projections -------------------
                with (
                    tc.tile_pool(name="wqkv", bufs=1) as wpool,
                    tc.tile_pool(name="hid", bufs=3) as hidp,
                    tc.tile_pool(name="ppsum", bufs=1, space="PSUM") as ppsum,
                    tc.tile_pool(name="vtmp", bufs=1) as vtmp,
                    tc.tile_pool(name="tppsum", bufs=2, space="PSUM") as tppsum,
                ):
                    # weight tiles are loaded per-d, interleaved with the first
                    # chunk's hid tiles so the first matmuls start after ~3
                    # small DMAs instead of the full 12MB
                    wq_tile = wpool.tile([128, ND, QD], BF16)
                    wkv_tile = wpool.tile([128, ND, KVD], BF16)
                    vT = vtmp.tile([128, S], BF16)  # [hd, sk], pre-transpose

                    DG = 8
                    for ch_i, ch in enumerate(A_ORDER):
                        cs = slice(ch * CH, (ch + 1) * CH)
                        psums = [ppsum.tile([128, CH], F32, tag=f"pp{m}", name=f"pp{m}") for m in range(6)]
                        for grp in range(ND // DG):
                            if ch_i == 0 and grp == 1:
                                _load_big_consts()
                            d0 = grp * DG
                            # batched DMAs: 2 hid half-group loads + 1 per
                            # weight tensor per group (SP issue rate is the
                            # DMA bottleneck, not bandwidth)
                            ht_g = hidp.tile([128, DG, CH], BF16, tag="hid")
                            for half in range(2):
                                hd0 = d0 + half * (DG // 2)
                                nc.sync.dma_start(
                                    out=ht_g[:, half * (DG // 2) : (half + 1) * (DG // 2), :],
                                    in_=hidT[
                                        hd0 * 128 : (hd0 + DG // 2) * 128, cs
                                    ].rearrange("(g p) m -> p g m", p=128),
                                )
                            if ch_i == 0:
                                for half in range(2):
                                    hd0 = d0 + half * (DG // 2)
                                    nc.sync.dma_start(
                                        out=wq_tile[:, hd0 : hd0 + DG // 2, :],
                                        in_=wq_t[
                                            hd0 * 128 : (hd0 + DG // 2) * 128, :
                                        ].rearrange("(g p) m -> p g m", p=128),
                                    )
                                    nc.sync.dma_start(
                                        out=wkv_tile[:, hd0 : hd0 + DG // 2, :],
                                        in_=wkv_t[
                                            hd0 * 128 : (hd0 + DG // 2) * 128, :
                                        ].rearrange("(g p) m -> p g m", p=128),
                                    )
                            # d-major: the first 6 matmuls need only the first
                            # half-group hid DMA + this group's weight DMAs
                            for dl in range(DG):
                                d = d0 + dl
                                for m in range(6):
                                    if m < NH_LOC:
                                        w_ap = wq_tile[:, :, m * 128 : (m + 1) * 128]
                                    else:
                                        mm = m - NH_LOC
                                        w_ap = wkv_tile[:, :, mm * 128 : (mm + 1) * 128]
                                    nc.tensor.matmul(
                                        psums[m][:, :],
                                        w_ap[:, d, :],
                                        ht_g[:, dl, :],
                                        start=(d == 0),
                                        stop=(d == ND - 1),
                                        skip_group_check=True,
                                    )
                        # bias copies split across DVE/ACT so all 6 psums
                        # free within ~1.5us; rope rotations follow on Pool
                        nc.vector.tensor_scalar_add(qT_ch[ch][:, 0, :], psums[0], bq_t[:, 0:1])
                        nc.vector.tensor_scalar_add(qT_ch[ch][:, 1, :], psums[1], bq_t[:, 1:2])
                        nc.scalar.activation(qT_ch[ch][:, 2, :], psums[2], AF.Identity, bias=bq_t[:, 2:3])
                        nc.scalar.activation(qT_ch[ch][:, 3, :], psums[3], AF.Identity, bias=bq_t[:, 3:4])
                        nc.scalar.activation(kT_ch[ch][:, :], psums[4], AF.Identity, bias=bkv_t[:, 0:1])
                        nc.scalar.activation(vT[:, cs], psums[5], AF.Identity, bias=bkv_t[:, 1:2])
                        for m in range(NH_LOC):
                            _rope_rot(nc, ropetmp, qT_ch[ch][:, m, :], cos_t[:, cs], sin_t[:, cs])
                        _rope_rot(nc, ropetmp, kT_ch[ch][:, :], cos_t[:, cs], sin_t[:, cs])

                        # transpose this chunk's v: vT [hd, sk] -> vN [sk, hd]
                        for tl in range(CH // 128):
                            t = ch * (CH // 128) + tl
                            pst = tppsum.tile([128, 128], BF16)
                            nc.tensor.transpose(
                                pst[:, :], vT[:, t * 128 : (t + 1) * 128], ident_t[:, :]
                            )
                            nc.vector.tensor_copy(vN_ch[ch][:, tl, :], pst[:, :])

                # phase B + C SBUF pools open together: the wo_t load streams
                # during attention, hiding the 8MB transfer
                with (
                    tc.tile_pool(name="wo", bufs=1) as wop,
                    tc.tile_pool(name="strip", bufs=3) as stripp,
                    tc.tile_pool(name="outp", bufs=3) as outp,
                ):
                    wo_tile = wop.tile([128, ND, QD], BF16)
                    for d in range(ND):
                        nc.sync.dma_start(
                            out=wo_tile[:, d, :],
                            in_=wo_t[d * 128 : (d + 1) * 128, :],
                        )

                    # ---------------- phase B: attention ---------------------
                    with (
                        tc.tile_pool(name="sppair", bufs=2, space="PSUM") as sppair,
                        tc.tile_pool(name="spdiag", bufs=2, space="PSUM") as spdiag,
                        tc.tile_pool(name="opsum", bufs=2, space="PSUM") as opsum,
                        tc.tile_pool(name="ptile", bufs=3) as ptile,
                        tc.tile_pool(name="accp", bufs=2) as accp,
                        tc.tile_pool(name="btmp", bufs=2) as btmp,
                    ):
                        ch_order = B_ORDER if causal else list(range(NCH))
                        for ch in ch_order:
                            cs = slice(ch * CH, (ch + 1) * CH)
                            nt = 4 * (ch + 1) if causal else NT
                            n_full = nt - 4 if causal else NT
                            for h in range(NH_LOC):
                                ps_o = opsum.tile([128, CH], F32, tag="o")
                                acc = accp.tile([128, CH], F32R, tag="acc")
                                first = True
                                # full tiles, processed in pairs sharing one
                                # 2-bank psum tile -> single exp instruction
                                for pr in range(n_full // 2):
                                    t0, t1 = 2 * pr, 2 * pr + 1
                                    ps2 = sppair.tile([128, 2, CH], F32, tag="sc2")
                                    for ti, t in enumerate((t0, t1)):
                                        nc.tensor.matmul(
                                            ps2[:, ti, :],
                                            kT_ch[t // 4][:, (t % 4) * 128 : (t % 4 + 1) * 128],
                                            qT_ch[ch][:, h, :],
                                            start=True,
                                            stop=True,
                                        )
                                    p2 = ptile.tile([128, 2, CH], BF16, tag="p2")
                                    if causal:
                                        nc.scalar.activation(
                                            p2[:, :, :], ps2[:, :, :], AF.Exp, scale=SCALE
                                        )
                                    else:
                                        for ti, t in enumerate((t0, t1)):
                                            mt = btmp.tile([128, CH], F32, tag="mt")
                                            nc.sync.dma_start(
                                                out=mt,
                                                in_=maskT[t * 128 : (t + 1) * 128, cs],
                                            )
                                            nc.vector.scalar_tensor_tensor(
                                                ps2[:, ti, :], ps2[:, ti, :], 1.0,
                                                mt[:, :], op0=ALU.mult, op1=ALU.add,
                                            )
                                        nc.scalar.activation(
                                            p2[:, :, :], ps2[:, :, :], AF.Exp, scale=SCALE
                                        )
                                    nc.tensor.matmul(
                                        ps_o[:, :], vN_ch[t0 // 4][:, t0 % 4, :], p2[:, 0, :],
                                        start=first, stop=False, skip_group_check=True,
                                    )
                                    nc.tensor.matmul(
                                        ps_o[:, :], vN_ch[t1 // 4][:, t1 % 4, :], p2[:, 1, :],
                                        start=False, stop=(not causal and pr == n_full // 2 - 1),
                                        skip_group_check=True,
                                    )
                                    # Pool-side softmax denominator accumulation
                                    if first:
                                        nc.gpsimd.tensor_add(
                                            acc[:, :], p2[:, 0, :], p2[:, 1, :]
                                        )
                                        first = False
                                    else:
                                        nc.gpsimd.tensor_add(acc[:, :], acc[:, :], p2[:, 0, :])
                                        nc.gpsimd.tensor_add(acc[:, :], acc[:, :], p2[:, 1, :])
                                # diagonal tiles (causal only)
                                if causal:
                                    for j in range(4):
                                        t = n_full + j
                                        sq0 = 128 * j  # valid-data start
                                        sq0m = min(sq0, 256)  # f32r >=256 matmul start
                                        # chunk 0 has no pair tiles: borrow the
                                        # idle pair-pool buffers (same tag ->
                                        # same banks) for a 4-deep diag pipeline
                                        if n_full == 0 and j % 2 == 1:
                                            psd = sppair.tile([128, CH], F32, tag="sc2")
                                        else:
                                            psd = spdiag.tile([128, CH], F32, tag="scd")
                                        nc.tensor.matmul(
                                            psd[:, sq0m:],
                                            kT_ch[t // 4][:, (t % 4) * 128 : (t % 4 + 1) * 128],
                                            qT_ch[ch][:, h, sq0m:],
                                            start=True,
                                            stop=True,
                                        )
                                        # mask only the 128-wide diagonal block,
                                        # in place (mask pre-divided by SCALE)
                                        nc.vector.scalar_tensor_tensor(
                                            psd[:, sq0 : sq0 + 128],
                                            psd[:, sq0 : sq0 + 128],
                                            1.0,
                                            mask_t[:, j, sq0 : sq0 + 128],
                                            op0=ALU.mult,
                                            op1=ALU.add,
                                        )
                                        pd = ptile.tile([128, CH], BF16, tag="pd")
                                        nc.scalar.activation(
                                            pd[:, sq0:], psd[:, sq0:], AF.Exp, scale=SCALE
                                        )
                                        nc.tensor.matmul(
                                            ps_o[:, sq0:], vN_ch[t // 4][:, t % 4, :], pd[:, sq0:],
                                            start=first, stop=(j == 3),
                                            skip_group_check=True,
                                        )
                                        if first:
                                            nc.gpsimd.tensor_copy(acc[:, :], pd[:, :])
                                            first = False
                                        else:
                                            nc.gpsimd.tensor_add(
                                                acc[:, sq0:], acc[:, sq0:], pd[:, sq0:]
                                            )
                                # normalize: single short ones-matmul reduces
                                # the Pool-accumulated acc across partitions;
                                # reciprocal + PE broadcast + multiply.  The
                                # sums/bcast psums ride the diag pool's
                                # buffer rotation (no extra banks).
                                spool, stag = spdiag, "scd"
                                sums = spool.tile([1, CH], F32, tag=stag, name="sums")
                                nc.tensor.matmul(
                                    sums[:, :], ones_col_t[:, :], acc[:, :],
                                    start=True, stop=True,
                                )
                                rec = accp.tile([1, CH], F32R, tag="rec")
                                with nc.allow_low_precision(reason="f32r recip"):
                                    nc.vector.reciprocal(rec[:, :], sums[:, :])
                                ps_bc = spool.tile([128, CH], F32, tag=stag, name="bcp")
                                nc.tensor.matmul(
                                    ps_bc[:, :], ones_row_t[:, :], rec[:, :],
                                    start=True, stop=True,
                                )
                                bc_sb = btmp.tile([128, CH], F32R, tag="bcs")
                                nc.vector.tensor_copy(bc_sb[:, :], ps_bc[:, :])
                                an = btmp.tile([128, CH], BF16, tag="an")
                                nc.vector.tensor_mul(an[:, :], ps_o[:, :], bc_sb[:, :])
                                nc.sync.dma_start(
                                    out=ag_in[ch, h * 128 : (h + 1) * 128, :],
                                    in_=an[:, :],
                                )
                            # fire this sq-chunk's AllGather as soon as its 4
                            # heads are done; overlaps remaining attention +
                            # output projection
                            if not skip_collective:
                                nc.gpsimd.collective_compute(
                                    "AllGather",
                                    ALU.bypass,
                                    ins=[ag_in[ch, :, :]],
                                    outs=[ag_out[ch, :, :]],
                                    replica_groups=[list(range(N_CORES))],
                                )

                    # ---------------- phase C: output projection ----------
                    with tc.tile_pool(name="copsum", bufs=2, space="PSUM") as copsum:
                        ps_bo = copsum.tile([128, QD], F32, tag="co0", name="psbo")
                        nc.tensor.matmul(
                            ps_bo[:, :], ones_row_t[:, :], bo_t[:, :],
                            start=True, stop=True,
                        )
                        bo_bc = outp.tile([128, QD], F32, tag="bo")
                        nc.vector.tensor_copy(bo_bc[:, :], ps_bo[:, :])

                        sqb_order = B_ORDER if causal else list(range(NCH))
                        for sqb in sqb_order:
                            ps_outs = [
                                copsum.tile(
                                    [128, QD], F32, tag=f"co{j}", name=f"co{j}"
                                )
                                for j in range(4)
                            ]
                            SG = 4  # strips per batched DMA
                            for dg in range(ND // SG):
                                d0 = dg * SG
                                strip = stripp.tile([128, SG, CH], BF16, tag="strip")
                                nc.sync.dma_start(
                                    out=strip,
                                    in_=ag_out[
                                        sqb, d0 * 128 : (d0 + SG) * 128, :
                                    ].rearrange("(g p) m -> p g m", p=128),
                                )
                                for dl in range(SG):
                                    d = d0 + dl
                                    for j in range(4):
                                        nc.tensor.matmul(
                                            ps_outs[j][:, :],
                                            strip[:, dl, j * 128 : (j + 1) * 128],
                                            wo_tile[:, d, :],
                                            start=(d == 0),
                                            stop=(d == ND - 1),
                                            skip_group_check=True,
                                        )
                            for j in range(4):
                                ot = outp.tile([128, QD], F32, tag="ot")
                                nc.vector.tensor_add(
                                    ot[:, :], ps_outs[j][:, :], bo_bc[:, :]
                                )
                                nc.sync.dma_start(
                                    out=out[
                                        sqb * CH + j * 128 : sqb * CH + (j + 1) * 128,
                                        :,
                                    ],
                                    in_=ot[:, :],
                                )

    if not skip_waitfix:
        _split_waits(nc)
    return nc


def _rope_rot(nc, tmp, dst, cos_s, sin_s):
    """In-place partial rope on dst[hd, s] (f32r): rows 0:64 rotated, rest pass.

    cos_s [64, CH]: cos table duplicated over both 32-row halves.
    sin_s [64, CH]: sign-folded sin: rows 0:32 = -sin, rows 32:64 = +sin, so
      new[0:64] = x[0:64]*cos_s + rot32(x[0:64])*sin_s
    with rot32 = swap of the two 32-row halves (done via SBUF->SBUF DMA,
    since compute engines cannot cross partitions).  All SBUF-only, so the
    muls/add run on the otherwise-idle Pool(GpSimd) engine, keeping DVE
    free for the attention phase that follows.
    """
    R2 = ROT // 2
    qsh = tmp.tile([ROT, CH], F32R, tag="r1")
    # issue the swap DMAs from the ACT queue: idle during projections, and
    # keeps them out from behind the big hid/weight loads on the SP queue
    nc.scalar.dma_start(out=qsh[0:R2, :], in_=dst[R2:ROT, :])
    nc.scalar.dma_start(out=qsh[R2:ROT, :], in_=dst[0:R2, :])
    t1 = tmp.tile([ROT, CH], F32, tag="r2")
    nc.gpsimd.tensor_mul(t1[:, :], dst[0:ROT, :], cos_s[:, :])
    t2 = tmp.tile([ROT, CH], F32, tag="r3")
    nc.gpsimd.tensor_mul(t2[:, :], qsh[:, :], sin_s[:, :])
    nc.gpsimd.tensor_add(dst[0:ROT, :], t1[:, :], t2[:, :])


# --------------------------------------------------------------------------
# host side: shard, run, gather
# --------------------------------------------------------------------------
_NC_CACHE = {}


def _get_nc(causal: bool) -> bass.Bass:
    if causal not in _NC_CACHE:
        _NC_CACHE[causal] = build(causal)
    return _NC_CACHE[causal]


def _rope_tables():
    inv_freq = 1.0 / (BASE ** (np.arange(0, ROT, 2, dtype=np.float64) / ROT))
    t = np.arange(S, dtype=np.float64)
    freqs = np.outer(t, inv_freq)  # [S, 32]
    import ml_dtypes

    cos32 = np.cos(freqs).T.astype(np.float32)  # [32, S]
    sin32 = np.sin(freqs).T.astype(np.float32)
    cosT = np.concatenate([cos32, cos32], axis=0).astype(ml_dtypes.bfloat16)
    sinT = np.concatenate([-sin32, sin32], axis=0).astype(ml_dtypes.bfloat16)
    return cosT, sinT


def _check_causal(mask):
    """mask: [1,1,S,S]. True if it is exactly a causal additive mask."""
    m = mask[0, 0]
    if not (m[np.tril_indices(S)] == 0.0).all():
        return False
    iu = np.triu_indices(S, k=1)
    vals = m[iu]
    return bool((vals <= -1e30).all()) and bool((vals == vals[0]).all())


def _prescale_mask(m64):
    """Divide mask by SCALE in f64, clipped to the f32 range."""
    fmax = np.finfo(np.float32).max
    return np.clip(m64.astype(np.float64) / SCALE, -fmax, fmax).astype(np.float32)


def _make_in_maps(inputs, causal):
    import ml_dtypes
    hidden = np.asarray(inputs["hidden_states"], dtype=np.float32)
    mask = np.asarray(inputs["attention_mask"], dtype=np.float32)
    Wq = np.asarray(inputs["Wq"], dtype=np.float32)
    bq = np.asarray(inputs["bq"], dtype=np.float32)
    Wkv = np.asarray(inputs["Wkv"], dtype=np.float32)
    bkv = np.asarray(inputs["bkv"], dtype=np.float32)
    Wo = np.asarray(inputs["Wo"], dtype=np.float32)
    bo = np.asarray(inputs["bo"], dtype=np.float32)

    hidT_bf = np.ascontiguousarray(hidden[0].T).astype(ml_dtypes.bfloat16)  # [H, S]
    cosT, sinT = _rope_tables()
    ident = np.eye(128, dtype=ml_dtypes.bfloat16)
    ones_col = np.ones((128, 1), np.float32)
    ones_row = np.ones((1, 128), np.float32)

    if causal:
        # diagonal strips from the actual mask (chunk 0 is representative --
        # _check_causal guarantees the pattern is uniform along the diagonal)
        maskT = np.stack(
            [_prescale_mask(
                np.ascontiguousarray(mask[0, 0, 0:CH, 128 * j : 128 * j + 128].T))
             for j in range(4)]
        )  # [4, 128, CH]
    else:
        maskT = _prescale_mask(np.ascontiguousarray(mask[0, 0].T))  # [S, S]

    in_maps = []
    for c in range(N_CORES):
        qs = slice(c * QD, (c + 1) * QD)
        kvs = slice(c * KVD, (c + 1) * KVD)
        in_maps.append(
            {
                "hidT": hidT_bf,
                "wq_t": np.ascontiguousarray(Wq[qs, :].T).astype(ml_dtypes.bfloat16),
                "wkv_t": np.ascontiguousarray(Wkv[kvs, :].T).astype(ml_dtypes.bfloat16),
                "wo_t": np.ascontiguousarray(Wo[qs, :].T).astype(ml_dtypes.bfloat16),
                "bq": np.ascontiguousarray(bq[qs].reshape(NH_LOC, 128).T),
                "bkv": np.ascontiguousarray(
                    bkv[kvs].reshape(2, 128).T
                ),
                "bo": bo[qs].reshape(1, QD),
                "cosT": cosT,
                "sinT": sinT,
                "ident": ident,
                "ones_col": ones_col,
                "ones_row": ones_row,
                "maskT": maskT,
            }
        )
    return in_maps


def kernel(**inputs) -> np.ndarray:
    causal = _check_causal(np.asarray(inputs["attention_mask"], dtype=np.float32))
    nc = _get_nc(causal)
    in_maps = _make_in_maps(inputs, causal)
    res = run_bass_kernel_spmd(nc, in_maps, list(range(N_CORES)))
    outs = [res.results[c]["out"] for c in range(N_CORES)]  # each [S, QD]
    full = np.concatenate(outs, axis=1)  # [S, H]
    return full.reshape(B, S, H)


# revision 47
# speedup vs baseline: 1.0784x; 1.0270x over previous
"""Tensor-parallel GQA attention (CustomLlamaAttention) on 8 TRN2 NeuronCores.

Sharding: heads.  Core c owns Q heads 4c..4c+3 and KV head c.
  - Wq/Wkv output dims sharded; attention fully head-local per core.
  - Output projection sharded over Wo *rows* (output dim): each core computes
    out[:, 512c:512c+512] after an AllGather of the per-core attention
    outputs (transposed layout [hd, s]) -- cheaper than the all-reduce
    variant (4MB gather vs 32MB reduce).

Per-core dataflow (f32r q/k scores at full PE rate, bf16 p/v):
  hidT [4096,2048] -> Q/KV projections (d-major matmul order so the first
  matmuls only wait on 3 DMAs) -> qT/kT [hd,s] + partial RoPE (bias copies
  split across DVE/ACT/Pool so the PSUM banks free fast at chunk
  boundaries); vT -> PE-transpose -> vN [sk,hd] bf16.

  Attention per (chunk, head), [sk,sq] layout: paired score tiles share one
  2-bank PSUM tile so a single ACT exp covers both (halves ACT per-instr
  overhead); diagonal tiles get the additive mask (pre-divided by 1/sqrt(d)
  host-side) applied only to the 128-wide diagonal block, in place in PSUM.
  Softmax denominators NEVER touch the PE: the Pool(GpSimd) engine
  accumulates exp tiles (acc += p) and a partition_all_reduce replicates
  column sums to all partitions; DVE reciprocal+multiply normalizes.

  AllGather is chunked along sq (4 collectives), each fired as soon as its
  chunk's 4 heads are normalized, hiding collective latency behind the
  remaining attention chunks and the output projection.

  Phase order: projections [3,0,1,2], attention [2,3,0,1] so the first
  attention chunk has 8 unmasked score tiles of runway while the last
  projection chunk's rope stores drain on DVE.

Matmul/DMA instructions can carry only one semaphore wait on this
toolchain (single EVENTS slot in the ISA); waitfix splits excess waits
onto sequencer NOPs.
"""

import sys

sys.path.insert(0, "/opt/trn_rl_repo")

import numpy as np

import concourse.bass as bass
import concourse.mybir as mybir
import concourse.tile as tile
from concourse import bass_isa
from concourse.bass_utils import run_bass_kernel_spmd

# ---- problem constants (hardcoded per contract) ----
B, S, H = 1, 2048, 4096
NH, NKV, HD = 32, 8, 128
ROT = 64
BASE = 10000.0
N_CORES = 8
NH_LOC = NH // N_CORES  # 4 q heads per core
QD = NH_LOC * HD  # 512 local q dims
KVD = 2 * HD  # 256 local kv dims
CH = 512  # seq chunk (psum bank width in f32)
NCH = S // CH  # 4
ND = H // 128  # 32 contraction tiles
NT = S // 128  # 16 sk tiles
SCALE = 1.0 / float(np.sqrt(HD))

# natural chunk order both phases: attention chunk 0 only needs data from
# projection chunk 0 (done first), so phase B starts with zero stall
A_ORDER = [0, 1, 2, 3]
B_ORDER = [0, 1, 2, 3]

F32 = mybir.dt.float32
F32R = mybir.dt.float32r
BF16 = mybir.dt.bfloat16
AF = mybir.ActivationFunctionType
ALU = mybir.AluOpType


# --------------------------------------------------------------------------
# waitfix: split >1 semaphore waits per instruction onto sequencer NOPs
# --------------------------------------------------------------------------
def _split_waits(nc, max_waits=1):
    isa = nc.isa
    op = isa.Opcode.NEURON_ISA_TPB_OPCODE_NOP
    n_fixed = 0
    for f in nc.m.functions:
        for blk in f.blocks:
            il = blk.instructions
            fixes = []
            for i, inst in enumerate(il):
                si = inst.sync_info
                if si is None or len(si.on_wait) <= max_waits:
                    continue
                fixes.append((i, inst))
            for i, inst in reversed(fixes):
                si = inst.sync_info
                waits = list(si.on_wait)
                keep = waits[-max_waits:]
                nops = []
                for w in waits[:-max_waits]:
                    instr, fixups = bass_isa.isa_struct(isa, op, {})
                    nop = mybir.InstISA(
                        name=nc.get_next_instruction_name(),
                        isa_opcode=op.value,
                        engine=inst.engine,
                        instr=instr,
                        op_name="NOP",
                        ins=[],
                        outs=[],
                        ant_dict={},
                        verify=True,
                        ant_isa_is_sequencer_only=True,
                        ant_sbuf_fixups=fixups or None,
                    )
                    nop.sync_info = mybir.SyncInfo(on_wait=[w], on_update=[])
                    nops.append(nop)
                inst.sync_info = mybir.SyncInfo(on_wait=keep, on_update=si.on_update)
                for j, nop in enumerate(nops):
                    il.insert(i + j, nop)
                n_fixed += 1
    return n_fixed


# --------------------------------------------------------------------------
# kernel builder (SPMD program, same for all 8 cores)
# --------------------------------------------------------------------------
def build(causal: bool, skip_collective: bool = False, skip_waitfix: bool = False) -> bass.Bass:
    nc = bass.Bass()

    hidT = nc.declare_dram_parameter("hidT", [H, S], BF16, isOutput=False)
    wq_t = nc.declare_dram_parameter("wq_t", [H, QD], BF16, isOutput=False)
    wkv_t = nc.declare_dram_parameter("wkv_t", [H, KVD], BF16, isOutput=False)
    wo_t = nc.declare_dram_parameter("wo_t", [H, QD], BF16, isOutput=False)
    bq = nc.declare_dram_parameter("bq", [128, NH_LOC], F32, isOutput=False)
    bkv = nc.declare_dram_parameter("bkv", [128, 2], F32, isOutput=False)
    bo = nc.declare_dram_parameter("bo", [1, QD], F32R, isOutput=False)
    ones_col = nc.declare_dram_parameter("ones_col", [128, 1], F32R, isOutput=False)
    ones_row = nc.declare_dram_parameter("ones_row", [1, 128], F32R, isOutput=False)
    cosT = nc.declare_dram_parameter("cosT", [ROT, S], BF16, isOutput=False)
    sinT = nc.declare_dram_parameter("sinT", [ROT, S], BF16, isOutput=False)
    ident = nc.declare_dram_parameter("ident", [128, 128], BF16, isOutput=False)
    if causal:
        # 4 diagonal mask strips: strip j is the [sk 128, sq 512] transposed
        # mask block whose diagonal offset is 128*j.  Values pre-divided by
        # SCALE (clipped to f32 range) so exp's scale multiply re-applies it.
        maskT = nc.declare_dram_parameter("maskT", [4, 128, CH], F32, isOutput=False)
    else:
        maskT = nc.declare_dram_parameter("maskT", [S, S], F32, isOutput=False)
    out = nc.declare_dram_parameter("out", [S, QD], F32, isOutput=True)

    # chunk-major layout so each sq-chunk's collective operates on a
    # contiguous DRAM region
    ag_in = nc.dram_tensor("ag_in", [NCH, QD, CH], BF16)
    ag_out = nc.dram_tensor("ag_out", [NCH, H, CH], BF16, addr_space="Shared")

    with tile.TileContext(nc) as tc:
        with tc.tile_pool(name="consts", bufs=1) as consts:
            # all const tables are DMA'd after chunk 0's first d-group so
            # they don't delay the first matmuls
            bq_t = consts.tile([128, NH_LOC], F32)
            bkv_t = consts.tile([128, 2], F32)
            ident_t = consts.tile([128, 128], BF16)
            cos_t = consts.tile([ROT, S], BF16)
            sin_t = consts.tile([ROT, S], BF16)
            bo_t = consts.tile([1, QD], F32R)
            ones_col_t = consts.tile([128, 1], F32R)
            ones_row_t = consts.tile([1, 128], F32R)
            mask_t = consts.tile([128, 4, CH], F32, name="mask_t") if causal else None

            def _load_big_consts():
                nc.scalar.dma_start(out=ones_col_t, in_=ones_col[:, :])
                nc.scalar.dma_start(out=ones_row_t, in_=ones_row[:, :])
                nc.scalar.dma_start(out=bq_t, in_=bq[:, :])
                nc.scalar.dma_start(out=bkv_t, in_=bkv[:, :])
                nc.scalar.dma_start(out=cos_t, in_=cosT[:, :])
                nc.scalar.dma_start(out=sin_t, in_=sinT[:, :])
                nc.scalar.dma_start(out=ident_t, in_=ident[:, :])
                nc.scalar.dma_start(out=bo_t, in_=bo[:, :])
                if causal:
                    nc.scalar.dma_start(
                        out=mask_t, in_=maskT.rearrange("j p m -> p j m")
                    )

            with (
                tc.tile_pool(name="qkv", bufs=1) as qkv,
                # rope scratch lives past phase A so phase-B pools don't
                # alias memory still being read by the rope chain's tail
                tc.tile_pool(name="ropetmp", bufs=2) as ropetmp,
            ):
                # per-chunk tiles: dependency tracking is per-tile, so the
                # attention phase's chunk-0 reads only wait on projection
                # chunk 0, not on the last chunk's rope chain
                qT_ch = [qkv.tile([128, NH_LOC, CH], F32R, name=f"qT{c}") for c in range(NCH)]
                kT_ch = [qkv.tile([128, CH], F32R, name=f"kT{c}") for c in range(NCH)]
                vN_ch = [qkv.tile([128, CH // 128, HD], BF16, name=f"vN{c}") for c in range(NCH)]

                # ---------------- phase A: projections -------------------
                with (
                    tc.tile_pool(name="wqkv", bufs=1) as wpool,
                    tc.tile_pool(name="hid", bufs=3) as hidp,
                    tc.tile_pool(name="ppsum", bufs=1, space="PSUM") as ppsum,
                    tc.tile_pool(name="vtmp", bufs=1) as vtmp,
                    tc.tile_pool(name="tppsum", bufs=2, space="PSUM") as tppsum,
                ):
                    # weight tiles are loaded per-d, interleaved with the first
                    # chunk's hid tiles so the first matmuls start after ~3
                    # small DMAs instead of the full 12MB
                    wq_tile = wpool.tile([128, ND, QD], BF16)
                    wkv_tile = wpool.tile([128, ND, KVD], BF16)
                    vT = vtmp.tile([128, S], BF16)  # [hd, sk], pre-transpose

                    DG = 8
                    for ch_i, ch in enumerate(A_ORDER):
                        cs = slice(ch * CH, (ch + 1) * CH)
                        psums = [ppsum.tile([128, CH], F32, tag=f"pp{m}", name=f"pp{m}") for m in range(6)]
                        for grp in range(ND // DG):
                            if ch_i == 0 and grp == 2:
                                _load_big_consts()
                            d0 = grp * DG
                            # batched half-group DMAs; hid rides the SP pipe,
                            # weights the ACT pipe (each DMA occupies its
                            # issuing engine for the whole transfer, so the
                            # two streams must not share a queue)
                            ht_g = hidp.tile([128, DG, CH], BF16, tag="hid")
                            for half in range(2):
                                hd0 = d0 + half * (DG // 2)
                                nc.sync.dma_start(
                                    out=ht_g[:, half * (DG // 2) : (half + 1) * (DG // 2), :],
                                    in_=hidT[
                                        hd0 * 128 : (hd0 + DG // 2) * 128, cs
                                    ].rearrange("(g p) m -> p g m", p=128),
                                )
                                if ch_i == 0:
                                    nc.scalar.dma_start(
                                        out=wq_tile[:, hd0 : hd0 + DG // 2, :],
                                        in_=wq_t[
                                            hd0 * 128 : (hd0 + DG // 2) * 128, :
                                        ].rearrange("(g p) m -> p g m", p=128),
                                    )
                                    nc.scalar.dma_start(
                                        out=wkv_tile[:, hd0 : hd0 + DG // 2, :],
                                        in_=wkv_t[
                                            hd0 * 128 : (hd0 + DG // 2) * 128, :
                                        ].rearrange("(g p) m -> p g m", p=128),
                                    )
                            # d-major: the first 6 matmuls need only the first
                            # half-group hid DMA + this group's weight DMAs
                            for dl in range(DG):
                                d = d0 + dl
                                for m in range(6):
                                    if m < NH_LOC:
                                        w_ap = wq_tile[:, :, m * 128 : (m + 1) * 128]
                                    else:
                                        mm = m - NH_LOC
                                        w_ap = wkv_tile[:, :, mm * 128 : (mm + 1) * 128]
                                    nc.tensor.matmul(
                                        psums[m][:, :],
                                        w_ap[:, d, :],
                                        ht_g[:, dl, :],
                                        start=(d == 0),
                                        stop=(d == ND - 1),
                                        skip_group_check=True,
                                    )
                        # bias copies split across DVE/ACT so all 6 psums
                        # free within ~1.5us; rope rotations follow on Pool
                        nc.vector.tensor_scalar_add(qT_ch[ch][:, 0, :], psums[0], bq_t[:, 0:1])
                        nc.vector.tensor_scalar_add(qT_ch[ch][:, 1, :], psums[1], bq_t[:, 1:2])
                        nc.scalar.activation(qT_ch[ch][:, 2, :], psums[2], AF.Identity, bias=bq_t[:, 2:3])
                        nc.scalar.activation(qT_ch[ch][:, 3, :], psums[3], AF.Identity, bias=bq_t[:, 3:4])
                        nc.scalar.activation(kT_ch[ch][:, :], psums[4], AF.Identity, bias=bkv_t[:, 0:1])
                        nc.scalar.activation(vT[:, cs], psums[5], AF.Identity, bias=bkv_t[:, 1:2])
                        for m in range(NH_LOC):
                            _rope_rot(nc, ropetmp, qT_ch[ch][:, m, :], cos_t[:, cs], sin_t[:, cs])
                        _rope_rot(nc, ropetmp, kT_ch[ch][:, :], cos_t[:, cs], sin_t[:, cs])

                        # transpose this chunk's v: vT [hd, sk] -> vN [sk, hd]
                        for tl in range(CH // 128):
                            t = ch * (CH // 128) + tl
                            pst = tppsum.tile([128, 128], BF16)
                            nc.tensor.transpose(
                                pst[:, :], vT[:, t * 128 : (t + 1) * 128], ident_t[:, :]
                            )
                            nc.vector.tensor_copy(vN_ch[ch][:, tl, :], pst[:, :])

                # phase B + C SBUF pools open together: the wo_t load streams
                # during attention, hiding the 8MB transfer
                with (
                    tc.tile_pool(name="wo", bufs=1) as wop,
                    tc.tile_pool(name="strip", bufs=3) as stripp,
                    tc.tile_pool(name="outp", bufs=3) as outp,
                ):
                    wo_tile = wop.tile([128, ND, QD], BF16)
                    for d in range(ND):
                        nc.sync.dma_start(
                            out=wo_tile[:, d, :],
                            in_=wo_t[d * 128 : (d + 1) * 128, :],
                        )

                    # ---------------- phase B: attention ---------------------
                    with (
                        tc.tile_pool(name="sppair", bufs=2, space="PSUM") as sppair,
                        tc.tile_pool(name="spdiag", bufs=2, space="PSUM") as spdiag,
                        tc.tile_pool(name="opsum", bufs=2, space="PSUM") as opsum,
                        tc.tile_pool(name="ptile", bufs=3) as ptile,
                        tc.tile_pool(name="accp", bufs=2) as accp,
                        tc.tile_pool(name="btmp", bufs=2) as btmp,
                    ):
                        ch_order = B_ORDER if causal else list(range(NCH))
                        for ch in ch_order:
                            cs = slice(ch * CH, (ch + 1) * CH)
                            nt = 4 * (ch + 1) if causal else NT
                            n_full = nt - 4 if causal else NT
                            for h in range(NH_LOC):
                                ps_o = opsum.tile([128, CH], F32, tag="o")
                                acc = accp.tile([128, CH], F32R, tag="acc")
                                first = True
                                # full tiles, processed in pairs sharing one
                                # 2-bank psum tile -> single exp instruction
                                for pr in range(n_full // 2):
                                    t0, t1 = 2 * pr, 2 * pr + 1
                                    ps2 = sppair.tile([128, 2, CH], F32, tag="sc2")
                                    for ti, t in enumerate((t0, t1)):
                                        nc.tensor.matmul(
                                            ps2[:, ti, :],
                                            kT_ch[t // 4][:, (t % 4) * 128 : (t % 4 + 1) * 128],
                                            qT_ch[ch][:, h, :],
                                            start=True,
                                            stop=True,
                                        )
                                    p2 = ptile.tile([128, 2, CH], BF16, tag="p2")
                                    if causal:
                                        nc.scalar.activation(
                                            p2[:, :, :], ps2[:, :, :], AF.Exp, scale=SCALE
                                        )
                                    else:
                                        for ti, t in enumerate((t0, t1)):
                                            mt = btmp.tile([128, CH], F32, tag="mt")
                                            nc.sync.dma_start(
                                                out=mt,
                                                in_=maskT[t * 128 : (t + 1) * 128, cs],
                                            )
                                            nc.vector.scalar_tensor_tensor(
                                                ps2[:, ti, :], ps2[:, ti, :], 1.0,
                                                mt[:, :], op0=ALU.mult, op1=ALU.add,
                                            )
                                        nc.scalar.activation(
                                            p2[:, :, :], ps2[:, :, :], AF.Exp, scale=SCALE
                                        )
                                    nc.tensor.matmul(
                                        ps_o[:, :], vN_ch[t0 // 4][:, t0 % 4, :], p2[:, 0, :],
                                        start=first, stop=False, skip_group_check=True,
                                    )
                                    nc.tensor.matmul(
                                        ps_o[:, :], vN_ch[t1 // 4][:, t1 % 4, :], p2[:, 1, :],
                                        start=False, stop=(not causal and pr == n_full // 2 - 1),
                                        skip_group_check=True,
                                    )
                                    # Pool-side softmax denominator accumulation
                                    if first:
                                        nc.gpsimd.tensor_add(
                                            acc[:, :], p2[:, 0, :], p2[:, 1, :]
                                        )
                                        first = False
                                    else:
                                        nc.gpsimd.tensor_add(acc[:, :], acc[:, :], p2[:, 0, :])
                                        nc.gpsimd.tensor_add(acc[:, :], acc[:, :], p2[:, 1, :])
                                # diagonal tiles (causal only)
                                if causal:
                                    for j in range(4):
                                        t = n_full + j
                                        sq0 = 128 * j  # valid-data start
                                        sq0m = min(sq0, 256)  # f32r >=256 matmul start
                                        # chunk 0 has no pair tiles: borrow the
                                        # idle pair-pool buffers (same tag ->
                                        # same banks) for a 4-deep diag pipeline
                                        if n_full == 0 and j % 2 == 1:
                                            psd = sppair.tile([128, CH], F32, tag="sc2")
                                        else:
                                            psd = spdiag.tile([128, CH], F32, tag="scd")
                                        nc.tensor.matmul(
                                            psd[:, sq0m:],
                                            kT_ch[t // 4][:, (t % 4) * 128 : (t % 4 + 1) * 128],
                                            qT_ch[ch][:, h, sq0m:],
                                            start=True,
                                            stop=True,
                                        )
                                        # mask only the 128-wide diagonal block,
                                        # in place (mask pre-divided by SCALE)
                                        nc.vector.scalar_tensor_tensor(
                                            psd[:, sq0 : sq0 + 128],
                                            psd[:, sq0 : sq0 + 128],
                                            1.0,
                                            mask_t[:, j, sq0 : sq0 + 128],
                                            op0=ALU.mult,
                                            op1=ALU.add,
                                        )
                                        pd = ptile.tile([128, CH], BF16, tag="pd")
                                        nc.scalar.activation(
                                            pd[:, sq0:], psd[:, sq0:], AF.Exp, scale=SCALE
                                        )
                                        nc.tensor.matmul(
                                            ps_o[:, sq0:], vN_ch[t // 4][:, t % 4, :], pd[:, sq0:],
                                            start=first, stop=(j == 3),
                                            skip_group_check=True,
                                        )
                                        if first:
                                            nc.gpsimd.tensor_copy(acc[:, :], pd[:, :])
                                            first = False
                                        else:
                                            nc.gpsimd.tensor_add(
                                                acc[:, sq0:], acc[:, sq0:], pd[:, sq0:]
                                            )
                                # normalize: single short ones-matmul reduces
                                # the Pool-accumulated acc across partitions;
                                # reciprocal + PE broadcast + multiply.  The
                                # sums/bcast psums ride the diag pool's
                                # buffer rotation (no extra banks).
                                spool, stag = spdiag, "scd"
                                sums = spool.tile([1, CH], F32, tag=stag, name="sums")
                                nc.tensor.matmul(
                                    sums[:, :], ones_col_t[:, :], acc[:, :],
                                    start=True, stop=True,
                                )
                                rec = accp.tile([1, CH], F32R, tag="rec")
                                with nc.allow_low_precision(reason="f32r recip"):
                                    nc.vector.reciprocal(rec[:, :], sums[:, :])
                                ps_bc = spool.tile([128, CH], F32, tag=stag, name="bcp")
                                nc.tensor.matmul(
                                    ps_bc[:, :], ones_row_t[:, :], rec[:, :],
                                    start=True, stop=True,
                                )
                                bc_sb = btmp.tile([128, CH], F32R, tag="bcs")
                                nc.vector.tensor_copy(bc_sb[:, :], ps_bc[:, :])
                                an = btmp.tile([128, CH], BF16, tag="an")
                                nc.vector.tensor_mul(an[:, :], ps_o[:, :], bc_sb[:, :])
                                nc.sync.dma_start(
                                    out=ag_in[ch, h * 128 : (h + 1) * 128, :],
                                    in_=an[:, :],
                                )
                            # fire this sq-chunk's AllGather as soon as its 4
                            # heads are done; overlaps remaining attention +
                            # output projection
                            if not skip_collective:
                                nc.gpsimd.collective_compute(
                                    "AllGather",
                                    ALU.bypass,
                                    ins=[ag_in[ch, :, :]],
                                    outs=[ag_out[ch, :, :]],
                                    replica_groups=[list(range(N_CORES))],
                                )

                    # ---------------- phase C: output projection ----------
                    with tc.tile_pool(name="copsum", bufs=2, space="PSUM") as copsum:
                        ps_bo = copsum.tile([128, QD], F32, tag="co0", name="psbo")
                        nc.tensor.matmul(
                            ps_bo[:, :], ones_row_t[:, :], bo_t[:, :],
                            start=True, stop=True,
                        )
                        bo_bc = outp.tile([128, QD], F32, tag="bo")
                        nc.vector.tensor_copy(bo_bc[:, :], ps_bo[:, :])

                        sqb_order = B_ORDER if causal else list(range(NCH))
                        for sqb in sqb_order:
                            ps_outs = [
                                copsum.tile(
                                    [128, QD], F32, tag=f"co{j}", name=f"co{j}"
                                )
                                for j in range(4)
                            ]
                            SG = 4  # strips per batched DMA
                            for dg in range(ND // SG):
                                d0 = dg * SG
                                strip = stripp.tile([128, SG, CH], BF16, tag="strip")
                                # alternate the strip loads between the idle
                                # Pool SWDGE and the SP pipe
                                dma_eng = nc.gpsimd if dg % 2 == 0 else nc.sync
                                dma_eng.dma_start(
                                    out=strip,
                                    in_=ag_out[
                                        sqb, d0 * 128 : (d0 + SG) * 128, :
                                    ].rearrange("(g p) m -> p g m", p=128),
                                )
                                for dl in range(SG):
                                    d = d0 + dl
                                    for j in range(4):
                                        nc.tensor.matmul(
                                            ps_outs[j][:, :],
                                            strip[:, dl, j * 128 : (j + 1) * 128],
                                            wo_tile[:, d, :],
                                            start=(d == 0),
                                            stop=(d == ND - 1),
                                            skip_group_check=True,
                                        )
                            for j in range(4):
                                ot = outp.tile([128, QD], F32, tag="ot")
                                nc.vector.tensor_add(
                                    ot[:, :], ps_outs[j][:, :], bo_bc[:, :]
                                )
                                nc.sync.dma_start(
                                    out=out[
                                        sqb * CH + j * 128 : sqb * CH + (j + 1) * 128,
                                        :,
                                    ],
                                    in_=ot[:, :],
                                )

    if not skip_waitfix:
        _split_waits(nc)
    return nc


def _rope_rot(nc, tmp, dst, cos_s, sin_s):
    """In-place partial rope on dst[hd, s] (f32r): rows 0:64 rotated, rest pass.

    cos_s [64, CH]: cos table duplicated over both 32-row halves.
    sin_s [64, CH]: sign-folded sin: rows 0:32 = -sin, rows 32:64 = +sin, so
      new[0:64] = x[0:64]*cos_s + rot32(x[0:64])*sin_s
    with rot32 = swap of the two 32-row halves (done via SBUF->SBUF DMA,
    since compute engines cannot cross partitions).  All SBUF-only, so the
    muls/add run on the otherwise-idle Pool(GpSimd) engine, keeping DVE
    free for the attention phase that follows.
    """
    R2 = ROT // 2
    qsh = tmp.tile([ROT, CH], F32R, tag="r1")
    # issue the swap DMAs from the ACT queue: idle during projections, and
    # keeps them out from behind the big hid/weight loads on the SP queue
    nc.scalar.dma_start(out=qsh[0:R2, :], in_=dst[R2:ROT, :])
    nc.scalar.dma_start(out=qsh[R2:ROT, :], in_=dst[0:R2, :])
    t1 = tmp.tile([ROT, CH], F32, tag="r2")
    nc.gpsimd.tensor_mul(t1[:, :], dst[0:ROT, :], cos_s[:, :])
    t2 = tmp.tile([ROT, CH], F32, tag="r3")
    nc.gpsimd.tensor_mul(t2[:, :], qsh[:, :], sin_s[:, :])
    nc.gpsimd.tensor_add(dst[0:ROT, :], t1[:, :], t2[:, :])


# --------------------------------------------------------------------------
# host side: shard, run, gather
# --------------------------------------------------------------------------
_NC_CACHE = {}


def _get_nc(causal: bool) -> bass.Bass:
    if causal not in _NC_CACHE:
        _NC_CACHE[causal] = build(causal)
    return _NC_CACHE[causal]


def _rope_tables():
    inv_freq = 1.0 / (BASE ** (np.arange(0, ROT, 2, dtype=np.float64) / ROT))
    t = np.arange(S, dtype=np.float64)
    freqs = np.outer(t, inv_freq)  # [S, 32]
    import ml_dtypes

    cos32 = np.cos(freqs).T.astype(np.float32)  # [32, S]
    sin32 = np.sin(freqs).T.astype(np.float32)
    cosT = np.concatenate([cos32, cos32], axis=0).astype(ml_dtypes.bfloat16)
    sinT = np.concatenate([-sin32, sin32], axis=0).astype(ml_dtypes.bfloat16)
    return cosT, sinT


def _check_causal(mask):
    """mask: [1,1,S,S]. True if it is exactly a causal additive mask."""
    m = mask[0, 0]
    if not (m[np.tril_indices(S)] == 0.0).all():
        return False
    iu = np.triu_indices(S, k=1)
    vals = m[iu]
    return bool((vals <= -1e30).all()) and bool((vals == vals[0]).all())


def _prescale_mask(m64):
    """Divide mask by SCALE in f64, clipped to the f32 range."""
    fmax = np.finfo(np.float32).max
    return np.clip(m64.astype(np.float64) / SCALE, -fmax, fmax).astype(np.float32)


def _make_in_maps(inputs, causal):
    import ml_dtypes
    hidden = np.asarray(inputs["hidden_states"], dtype=np.float32)
    mask = np.asarray(inputs["attention_mask"], dtype=np.float32)
    Wq = np.asarray(inputs["Wq"], dtype=np.float32)
    bq = np.asarray(inputs["bq"], dtype=np.float32)
    Wkv = np.asarray(inputs["Wkv"], dtype=np.float32)
    bkv = np.asarray(inputs["bkv"], dtype=np.float32)
    Wo = np.asarray(inputs["Wo"], dtype=np.float32)
    bo = np.asarray(inputs["bo"], dtype=np.float32)

    hidT_bf = np.ascontiguousarray(hidden[0].T).astype(ml_dtypes.bfloat16)  # [H, S]
    cosT, sinT = _rope_tables()
    ident = np.eye(128, dtype=ml_dtypes.bfloat16)
    ones_col = np.ones((128, 1), np.float32)
    ones_row = np.ones((1, 128), np.float32)

    if causal:
        # diagonal strips from the actual mask (chunk 0 is representative --
        # _check_causal guarantees the pattern is uniform along the diagonal)
        maskT = np.stack(
            [_prescale_mask(
                np.ascontiguousarray(mask[0, 0, 0:CH, 128 * j : 128 * j + 128].T))
             for j in range(4)]
        )  # [4, 128, CH]
    else:
        maskT = _prescale_mask(np.ascontiguousarray(mask[0, 0].T))  # [S, S]

    in_maps = []
    for c in range(N_CORES):
        qs = slice(c * QD, (c + 1) * QD)
        kvs = slice(c * KVD, (c + 1) * KVD)
        in_maps.append(
            {
                "hidT": hidT_bf,
                "wq_t": np.ascontiguousarray(Wq[qs, :].T).astype(ml_dtypes.bfloat16),
                "wkv_t": np.ascontiguousarray(Wkv[kvs, :].T).astype(ml_dtypes.bfloat16),
                "wo_t": np.ascontiguousarray(Wo[qs, :].T).astype(ml_dtypes.bfloat16),
                "bq": np.ascontiguousarray(bq[qs].reshape(NH_LOC, 128).T),
                "bkv": np.ascontiguousarray(
                    bkv[kvs].reshape(2, 128).T
                ),
                "bo": bo[qs].reshape(1, QD),
                "cosT": cosT,
                "sinT": sinT,
                "ident": ident,
                "ones_col": ones_col,
                "ones_row": ones_row,
                "maskT": maskT,
            }
        )
    return in_maps


def kernel(**inputs) -> np.ndarray:
    causal = _check_causal(np.asarray(inputs["attention_mask"], dtype=np.float32))
    nc = _get_nc(causal)
    in_maps = _make_in_maps(inputs, causal)
    res = run_bass_kernel_spmd(nc, in_maps, list(range(N_CORES)))
    outs = [res.results[c]["out"] for c in range(N_CORES)]  # each [S, QD]
    full = np.concatenate(outs, axis=1)  # [S, H]
    return full.reshape(B, S, H)
